# revision 10
# baseline (speedup 1.0000x reference)
"""Trainium2 Bass kernel for nn_DomainAdaptation (sparse feature-attention + dual MLP).

Math (reference):
    S = Q^T K                        [D, D], contraction over N
    L = exp(S - S*I/sqrt(D))
    scores = softmax(L, axis=-1)
    attn = (scores @ V^T)^T          [N, D]
    dom_q = relu(attn @ Wq1 + bq1) @ Wq2 + bq2
    dom_k = relu(attn @ Wk1 + bk1) @ Wk2 + bk2

Key restructuring: attn = V @ scores^T, so
    attn @ W1 = V @ (scores^T @ W1) = V @ M1
and attn is never materialized. Per core (N sharded 8 ways):
    phase 1: S_partial = Qc^T Kc  (bf16 matmuls, f32 accum)
    ReduceScatter(S) -> each core owns a 128-row block of S
    softmax block (f32, incl. double-exp of the reference), AllGather(scores)
    M1 = scores^T @ W1, h-sharded per core, AllGather(M1)  [x2 for q/k]
    MLP: hiddenT = relu(M1^T @ Vc^T + b1);  dom = hidden^T-contracted @ W2 + b2
outputs in natural [N, D] f32 orientation.
"""

import numpy as np
import ml_dtypes

N, D, H = 32768, 1024, 4096
NCORES = 8
NS = N // NCORES          # 4096 sample rows per core
HS = H // NCORES          # 512 hidden cols per core (M1 shard)
P = 128
BF = ml_dtypes.bfloat16

_CACHE: dict = {}


def _build():
    import concourse.bass as bass
    import concourse.tile as tile
    from concourse import bacc, mybir

    f32 = mybir.dt.float32
    bf16 = mybir.dt.bfloat16
    Exp = mybir.ActivationFunctionType.Exp
    add = mybir.AluOpType.add
    mx = mybir.AluOpType.max
    mult = mybir.AluOpType.mult

    nc = bacc.Bacc("TRN2", target_bir_lowering=False, debug=False, num_devices=NCORES)

    # ---- I/O ----
    q = nc.dram_tensor("q", [NS, D], bf16, kind="ExternalInput")
    k = nc.dram_tensor("k", [NS, D], bf16, kind="ExternalInput")
    vt = nc.dram_tensor("vt", [D, NS], bf16, kind="ExternalInput")
    w1s = {m: nc.dram_tensor(f"w1s_{m}", [D, HS], bf16, kind="ExternalInput") for m in "qk"}
    w2 = {m: nc.dram_tensor(f"w2_{m}", [H, D], bf16, kind="ExternalInput") for m in "qk"}
    b1t = {m: nc.dram_tensor(f"b1t_{m}", [P, H // P], f32, kind="ExternalInput") for m in "qk"}
    b2r = {m: nc.dram_tensor(f"b2r_{m}", [1, D], f32, kind="ExternalInput") for m in "qk"}
    mask = nc.dram_tensor("mask", [P, D], f32, kind="ExternalInput")
    dom = {m: nc.dram_tensor(f"dom_{m}", [NS, D], f32, kind="ExternalOutput") for m in "qk"}

    # ---- internal DRAM (collective bounce buffers) ----
    s_part = [nc.dram_tensor(f"s_part{j}", [D, 512], f32) for j in range(2)]
    s_red = [nc.dram_tensor(f"s_red{j}", [P, 512], f32) for j in range(2)]
    scb = nc.dram_tensor("scb", [P, D], bf16)
    sc_full = nc.dram_tensor("sc_full", [D, D], bf16, addr_space="Shared")
    m1s = {(m, h): nc.dram_tensor(f"m1s_{m}{h}", [D // 2, HS], bf16)
           for m in "qk" for h in range(2)}
    m1f = {(m, h): nc.dram_tensor(f"m1f_{m}{h}", [NCORES, D // 2, HS], bf16,
                                  addr_space="Shared")
           for m in "qk" for h in range(2)}

    RG = [list(range(NCORES))]
    NB = NS // P              # 32 n-blocks per core
    IT = D // P               # 8 feature tiles
    JW = 512                  # matmul moving free dim
    JH = D // JW              # 2 j-halves of S
    HB = H // P               # 32 hidden blocks
    KO = 4                    # phase-1 k-stream chunks (of NB//KO n-blocks each)
    NBC = NB // KO            # 8 n-blocks per stream chunk

    with tile.TileContext(nc) as tc:
        with (
            tc.tile_pool(name="small", bufs=1) as small,
            tc.tile_pool(name="dout", bufs=4) as doutp,
        ):
            mask_sb = small.tile([P, D], f32)
            nc.sync.dma_start(out=mask_sb[:], in_=mask.ap())

            # ================= phase 1: S_partial = Qc^T Kc =================
            with (
                tc.tile_pool(name="ph1", bufs=1) as ph1,
                tc.tile_pool(name="kstream", bufs=2) as kstream,
                tc.tile_pool(name="ph1psum", bufs=1, space="PSUM") as ph1psum,
            ):
                q_sb = ph1.tile([P, NB, D], bf16)
                for nb in range(NB):
                    nc.sync.dma_start(
                        out=q_sb[:, nb, :],
                        in_=q.ap()[nb * P:(nb + 1) * P, :],
                    )
                for jh in range(JH):
                    ps = [
                        ph1psum.tile([P, JW], f32, tag=f"sps{i}", name=f"sps{i}_{jh}")
                        for i in range(IT)
                    ]
                    for ko in range(KO):
                        k_sb = kstream.tile([P, NBC, JW], bf16, tag="kc")
                        nc.sync.dma_start(
                            out=k_sb[:],
                            in_=k.ap()[ko * NBC * P:(ko + 1) * NBC * P,
                                       jh * JW:(jh + 1) * JW]
                                .rearrange("(nb p) d -> p nb d", p=P),
                        )
                        for nb in range(NBC):
                            for i in range(IT):
                                nc.tensor.matmul(
                                    ps[i][:],
                                    q_sb[:, ko * NBC + nb, i * P:(i + 1) * P],
                                    k_sb[:, nb, :],
                                    start=(ko == 0 and nb == 0),
                                    stop=(ko == KO - 1 and nb == NBC - 1),
                                )
                    for i in range(IT):
                        so = doutp.tile([P, JW], f32, tag="sout")
                        nc.vector.tensor_copy(out=so[:], in_=ps[i][:])
                        nc.sync.dma_start(
                            out=s_part[jh].ap()[i * P:(i + 1) * P, :],
                            in_=so[:],
                        )
                    # ReduceScatter this column-half; the jh=0 one overlaps
                    # the jh=1 matmuls.
                    nc.gpsimd.collective_compute(
                        "ReduceScatter", add, replica_groups=RG,
                        ins=[s_part[jh].ap().opt()], outs=[s_red[jh].ap().opt()],
                    )

            # ================= softmax block (f32) =================
            with tc.tile_pool(name="smx", bufs=1) as smx:
                sred_sb = smx.tile([P, D], f32)
                for j in range(2):
                    nc.sync.dma_start(out=sred_sb[:, j * JW:(j + 1) * JW],
                                      in_=s_red[j].ap())
                tmask = smx.tile([P, D], f32)
                nc.vector.tensor_tensor(out=tmask[:], in0=sred_sb[:], in1=mask_sb[:],
                                        op=mult)
                logit = smx.tile([P, D], f32)
                nc.scalar.activation(out=logit[:], in_=tmask[:], func=Exp)
                e2 = smx.tile([P, D], f32)
                zsum = smx.tile([P, 1], f32)
                nc.scalar.activation(out=e2[:], in_=logit[:], func=Exp,
                                     accum_out=zsum[:])
                rz = smx.tile([P, 1], f32)
                nc.vector.reciprocal(rz[:], zsum[:])
                scb_sb = smx.tile([P, D], bf16)
                nc.vector.tensor_scalar(out=scb_sb[:], in0=e2[:], scalar1=rz[:],
                                        scalar2=None, op0=mult)
                nc.sync.dma_start(out=scb.ap(), in_=scb_sb[:])

            nc.gpsimd.collective_compute(
                "AllGather", mybir.AluOpType.bypass, replica_groups=RG,
                ins=[scb.ap().opt()], outs=[sc_full.ap().opt()],
            )

            # ================= M1 = scores^T @ W1 (h-shard) =================
            with (
                tc.tile_pool(name="m1pool", bufs=1) as m1pool,
                tc.tile_pool(name="m1psum", bufs=3, space="PSUM") as m1psum,
            ):
                sc_sb = m1pool.tile([P, IT, D], bf16)
                nc.sync.dma_start(
                    out=sc_sb[:],
                    in_=sc_full.ap().rearrange("(it p) j -> p it j", p=P),
                )
                for m in "qk":
                    w1_sb = m1pool.tile([P, IT, HS], bf16, tag=f"w1_{m}")
                    nc.sync.dma_start(
                        out=w1_sb[:],
                        in_=w1s[m].ap().rearrange("(it p) h -> p it h", p=P),
                    )
                    for half in range(2):
                        for jj in range(IT // 2):
                            jm = half * (IT // 2) + jj
                            mp = m1psum.tile([P, HS], f32, tag="m1ps",
                                             name=f"mp_{m}{jm}")
                            for it in range(IT):
                                nc.tensor.matmul(
                                    mp[:],
                                    sc_sb[:, it, jm * P:(jm + 1) * P],
                                    w1_sb[:, it, :],
                                    start=(it == 0),
                                    stop=(it == IT - 1),
                                )
                            mo = doutp.tile([P, HS], bf16, tag="m1out",
                                            name=f"mo_{m}{jm}")
                            nc.vector.tensor_copy(out=mo[:], in_=mp[:])
                            nc.sync.dma_start(
                                out=m1s[m, half].ap()[jj * P:(jj + 1) * P, :],
                                in_=mo[:],
                            )
                        nc.gpsimd.collective_compute(
                            "AllGather", mybir.AluOpType.bypass, replica_groups=RG,
                            ins=[m1s[m, half].ap().opt()],
                            outs=[m1f[m, half].ap().opt()],
                        )

            # ================= MLPs =================
            with (
                tc.tile_pool(name="mlp", bufs=1) as mlp,
                tc.tile_pool(name="vstream", bufs=2) as vstream,
                tc.tile_pool(name="mlppsum", bufs=5, space="PSUM") as bpsum,
                tc.tile_pool(name="cpsum", bufs=3, space="PSUM") as cpsum,
            ):
                for m in "qk":
                    m1_sb = mlp.tile([P, IT, H], bf16, tag="m1big")
                    for half in range(2):
                        for c2 in range(NCORES):
                            nc.sync.dma_start(
                                out=m1_sb[:, half * (IT // 2):(half + 1) * (IT // 2),
                                          c2 * HS:(c2 + 1) * HS],
                                in_=m1f[m, half].ap()[c2]
                                    .rearrange("(jb p) h -> p jb h", p=P),
                            )
                    w2_sb = mlp.tile([P, HB, D], bf16, tag="w2big")
                    nc.sync.dma_start(
                        out=w2_sb[:],
                        in_=w2[m].ap().rearrange("(hb p) d -> p hb d", p=P),
                    )
                    b1_sb = small.tile([P, H // P], f32, tag="b1t")
                    nc.sync.dma_start(out=b1_sb[:], in_=b1t[m].ap())
                    b2_sb = small.tile([P, D], f32, tag="b2r")
                    b2_bcast = b2r[m].ap()
                    nc.sync.dma_start(
                        out=b2_sb[:],
                        in_=bass.AP(tensor=b2_bcast.tensor, offset=b2_bcast.offset,
                                    ap=[[0, P], *b2_bcast.ap[1:]]),
                    )

                    for ncnk in range(NS // JW):      # 8 chunks of 512 samples
                        vt_sb = vstream.tile([P, IT, JW], bf16, tag="vt")
                        nc.sync.dma_start(
                            out=vt_sb[:],
                            in_=vt.ap()[:, ncnk * JW:(ncnk + 1) * JW]
                                .rearrange("(jb p) n -> p jb n", p=P),
                        )
                        hid_sb = mlp.tile([P, HB, JW], bf16, tag="hid")
                        # hiddenT[h, n] = relu(sum_j M1[j,h] vT[j,n] + b1[h])
                        for hb in range(HB):
                            pb = bpsum.tile([P, JW], f32, tag="psB")
                            for jb in range(IT):
                                nc.tensor.matmul(
                                    pb[:],
                                    m1_sb[:, jb, hb * P:(hb + 1) * P],
                                    vt_sb[:, jb, :],
                                    start=(jb == 0),
                                    stop=(jb == IT - 1),
                                )
                            nc.vector.tensor_scalar(
                                out=hid_sb[:, hb, :], in0=pb[:],
                                scalar1=b1_sb[:, hb:hb + 1], scalar2=0.0,
                                op0=add, op1=mx,
                            )
                        # dom[n, i2] = sum_h hidden[n,h] W2[h,i2] + b2[i2]
                        for ns in range(JW // P):     # 4 sample sub-tiles
                            for ih in range(JH):      # 2 output column halves
                                pc = cpsum.tile([P, JW], f32, tag="psC")
                                for hb in range(HB):
                                    nc.tensor.matmul(
                                        pc[:],
                                        hid_sb[:, hb, ns * P:(ns + 1) * P],
                                        w2_sb[:, hb, ih * JW:(ih + 1) * JW],
                                        start=(hb == 0), stop=(hb == HB - 1),
                                    )
                                do = doutp.tile([P, JW], f32, tag="dmout")
                                nc.vector.tensor_tensor(
                                    out=do[:], in0=pc[:],
                                    in1=b2_sb[:, ih * JW:(ih + 1) * JW],
                                    op=add,
                                )
                                nc.sync.dma_start(
                                    out=dom[m].ap()[
                                        ncnk * JW + ns * P:ncnk * JW + (ns + 1) * P,
                                        ih * JW:(ih + 1) * JW],
                                    in_=do[:],
                                )

    nc.compile()
    return nc


def _get_nc():
    if "nc" not in _CACHE:
        _CACHE["nc"] = _build()
    return _CACHE["nc"]


def _make_in_maps(inputs):
    query = np.asarray(inputs["query"])
    key = np.asarray(inputs["key"])
    value = np.asarray(inputs["value"])

    q_bf = query.astype(BF)
    k_bf = key.astype(BF)
    vt_bf = np.ascontiguousarray(value.T).astype(BF)          # [D, N]
    w1 = {"q": inputs["wq1"], "k": inputs["wk1"]}
    w2 = {"q": inputs["wq2"], "k": inputs["wk2"]}
    b1 = {"q": inputs["bq1"], "k": inputs["bk1"]}
    b2 = {"q": inputs["bq2"], "k": inputs["bk2"]}
    w1_bf = {m: np.asarray(w1[m]).astype(BF) for m in "qk"}
    w2_bf = {m: np.ascontiguousarray(np.asarray(w2[m]).astype(BF)) for m in "qk"}
    b1_t = {m: np.ascontiguousarray(
        np.asarray(b1[m]).astype(np.float32).reshape(H // P, P).T) for m in "qk"}
    b2_r = {m: np.asarray(b2[m]).astype(np.float32).reshape(1, D) for m in "qk"}

    in_maps = []
    diag = 1.0 - 1.0 / np.sqrt(D).astype(np.float32)
    for c in range(NCORES):
        msk = np.ones((P, D), np.float32)
        msk[np.arange(P), c * P + np.arange(P)] = diag
        im = {
            "q": np.ascontiguousarray(q_bf[c * NS:(c + 1) * NS]),
            "k": np.ascontiguousarray(k_bf[c * NS:(c + 1) * NS]),
            "vt": np.ascontiguousarray(vt_bf[:, c * NS:(c + 1) * NS]),
            "mask": msk,
        }
        for m in "qk":
            im[f"w1s_{m}"] = np.ascontiguousarray(
                w1_bf[m][:, c * HS:(c + 1) * HS])
            im[f"w2_{m}"] = w2_bf[m]
            im[f"b1t_{m}"] = b1_t[m]
            im[f"b2r_{m}"] = b2_r[m]
        in_maps.append(im)
    return in_maps


def _gather(results):
    dom_q = np.concatenate([results[c]["dom_q"] for c in range(NCORES)], axis=0)
    dom_k = np.concatenate([results[c]["dom_k"] for c in range(NCORES)], axis=0)
    return dom_q, dom_k


def _run(inputs, **kw):
    from concourse import bass_utils
    nc = _get_nc()
    in_maps = _make_in_maps(inputs)
    return bass_utils.run_bass_kernel_spmd(
        nc, in_maps, core_ids=list(range(NCORES)), **kw
    )


def kernel(**inputs):
    res = _run(inputs)
    return _gather(res.results)


# revision 11
# speedup vs baseline: 1.0027x; 1.0027x over previous
"""Trainium2 Bass kernel for nn_DomainAdaptation (sparse feature-attention + dual MLP).

Math (reference):
    S = Q^T K                        [D, D], contraction over N
    L = exp(S - S*I/sqrt(D))
    scores = softmax(L, axis=-1)
    attn = (scores @ V^T)^T          [N, D]
    dom_q = relu(attn @ Wq1 + bq1) @ Wq2 + bq2
    dom_k = relu(attn @ Wk1 + bk1) @ Wk2 + bk2

Key restructuring: attn = V @ scores^T, so
    attn @ W1 = V @ (scores^T @ W1) = V @ M1
and attn is never materialized. Per core (N sharded 8 ways):
    phase 1: S_partial = Qc^T Kc  (bf16 matmuls, f32 accum)
    ReduceScatter(S) -> each core owns a 128-row block of S
    softmax block (f32, incl. double-exp of the reference), AllGather(scores)
    M1 = scores^T @ W1, h-sharded per core, AllGather(M1)  [x2 for q/k]
    MLP: hiddenT = relu(M1^T @ Vc^T + b1);  dom = hidden^T-contracted @ W2 + b2
outputs in natural [N, D] f32 orientation.
"""

import numpy as np
import ml_dtypes

N, D, H = 32768, 1024, 4096
NCORES = 8
NS = N // NCORES          # 4096 sample rows per core
HS = H // NCORES          # 512 hidden cols per core (M1 shard)
P = 128
BF = ml_dtypes.bfloat16

_CACHE: dict = {}


def _build():
    import concourse.bass as bass
    import concourse.tile as tile
    from concourse import bacc, mybir

    f32 = mybir.dt.float32
    bf16 = mybir.dt.bfloat16
    Exp = mybir.ActivationFunctionType.Exp
    add = mybir.AluOpType.add
    mx = mybir.AluOpType.max
    mult = mybir.AluOpType.mult

    nc = bacc.Bacc("TRN2", target_bir_lowering=False, debug=False, num_devices=NCORES)

    # ---- I/O ----
    q = nc.dram_tensor("q", [NS, D], bf16, kind="ExternalInput")
    k = nc.dram_tensor("k", [NS, D], bf16, kind="ExternalInput")
    vt = nc.dram_tensor("vt", [D, NS], bf16, kind="ExternalInput")
    w1s = {m: nc.dram_tensor(f"w1s_{m}", [D, HS], bf16, kind="ExternalInput") for m in "qk"}
    w2 = {m: nc.dram_tensor(f"w2_{m}", [H, D], bf16, kind="ExternalInput") for m in "qk"}
    b1t = {m: nc.dram_tensor(f"b1t_{m}", [P, H // P], f32, kind="ExternalInput") for m in "qk"}
    b2r = {m: nc.dram_tensor(f"b2r_{m}", [1, D], f32, kind="ExternalInput") for m in "qk"}
    mask = nc.dram_tensor("mask", [P, D], f32, kind="ExternalInput")
    dom = {m: nc.dram_tensor(f"dom_{m}", [NS, D], f32, kind="ExternalOutput") for m in "qk"}

    # ---- internal DRAM (collective bounce buffers) ----
    s_part = [nc.dram_tensor(f"s_part{j}", [D, 512], f32) for j in range(2)]
    s_red = [nc.dram_tensor(f"s_red{j}", [P, 512], f32) for j in range(2)]
    scb = nc.dram_tensor("scb", [P, D], bf16)
    sc_full = nc.dram_tensor("sc_full", [D, D], bf16, addr_space="Shared")
    m1s = {(m, h): nc.dram_tensor(f"m1s_{m}{h}", [D // 2, HS], bf16)
           for m in "qk" for h in range(2)}
    m1f = {(m, h): nc.dram_tensor(f"m1f_{m}{h}", [NCORES, D // 2, HS], bf16,
                                  addr_space="Shared")
           for m in "qk" for h in range(2)}

    RG = [list(range(NCORES))]
    NB = NS // P              # 32 n-blocks per core
    IT = D // P               # 8 feature tiles
    JW = 512                  # matmul moving free dim
    JH = D // JW              # 2 j-halves of S
    HB = H // P               # 32 hidden blocks
    KO = 4                    # phase-1 k-stream chunks (of NB//KO n-blocks each)
    NBC = NB // KO            # 8 n-blocks per stream chunk

    with tile.TileContext(nc) as tc:
        with (
            tc.tile_pool(name="small", bufs=1) as small,
            tc.tile_pool(name="dout", bufs=4) as doutp,
        ):
            mask_sb = small.tile([P, D], f32)
            nc.sync.dma_start(out=mask_sb[:], in_=mask.ap())

            # ================= phase 1: S_partial = Qc^T Kc =================
            with (
                tc.tile_pool(name="ph1", bufs=1) as ph1,
                tc.tile_pool(name="kstream", bufs=2) as kstream,
                tc.tile_pool(name="ph1psum", bufs=1, space="PSUM") as ph1psum,
            ):
                q_ch = []
                for ko in range(KO):
                    qc = ph1.tile([P, NBC, D], bf16, tag=f"qc{ko}", name=f"qc{ko}")
                    nc.sync.dma_start(
                        out=qc[:],
                        in_=q.ap()[ko * NBC * P:(ko + 1) * NBC * P, :]
                            .rearrange("(nb p) d -> p nb d", p=P),
                    )
                    q_ch.append(qc)
                for jh in range(JH):
                    ps = [
                        ph1psum.tile([P, JW], f32, tag=f"sps{i}", name=f"sps{i}_{jh}")
                        for i in range(IT)
                    ]
                    for ko in range(KO):
                        k_sb = kstream.tile([P, NBC, JW], bf16, tag="kc")
                        nc.sync.dma_start(
                            out=k_sb[:],
                            in_=k.ap()[ko * NBC * P:(ko + 1) * NBC * P,
                                       jh * JW:(jh + 1) * JW]
                                .rearrange("(nb p) d -> p nb d", p=P),
                        )
                        for nb in range(NBC):
                            for i in range(IT):
                                nc.tensor.matmul(
                                    ps[i][:],
                                    q_ch[ko][:, nb, i * P:(i + 1) * P],
                                    k_sb[:, nb, :],
                                    start=(ko == 0 and nb == 0),
                                    stop=(ko == KO - 1 and nb == NBC - 1),
                                )
                    for i in range(IT):
                        so = doutp.tile([P, JW], f32, tag="sout")
                        nc.vector.tensor_copy(out=so[:], in_=ps[i][:])
                        nc.sync.dma_start(
                            out=s_part[jh].ap()[i * P:(i + 1) * P, :],
                            in_=so[:],
                        )
                    # ReduceScatter this column-half; the jh=0 one overlaps
                    # the jh=1 matmuls.
                    nc.gpsimd.collective_compute(
                        "ReduceScatter", add, replica_groups=RG,
                        ins=[s_part[jh].ap().opt()], outs=[s_red[jh].ap().opt()],
                    )

            # ================= softmax block (f32) =================
            with tc.tile_pool(name="smx", bufs=1) as smx:
                sred_sb = smx.tile([P, D], f32)
                for j in range(2):
                    nc.sync.dma_start(out=sred_sb[:, j * JW:(j + 1) * JW],
                                      in_=s_red[j].ap())
                tmask = smx.tile([P, D], f32)
                nc.vector.tensor_tensor(out=tmask[:], in0=sred_sb[:], in1=mask_sb[:],
                                        op=mult)
                logit = smx.tile([P, D], f32)
                nc.scalar.activation(out=logit[:], in_=tmask[:], func=Exp)
                e2 = smx.tile([P, D], f32)
                zsum = smx.tile([P, 1], f32)
                nc.scalar.activation(out=e2[:], in_=logit[:], func=Exp,
                                     accum_out=zsum[:])
                rz = smx.tile([P, 1], f32)
                nc.vector.reciprocal(rz[:], zsum[:])
                scb_sb = smx.tile([P, D], bf16)
                nc.vector.tensor_scalar(out=scb_sb[:], in0=e2[:], scalar1=rz[:],
                                        scalar2=None, op0=mult)
                nc.sync.dma_start(out=scb.ap(), in_=scb_sb[:])

            nc.gpsimd.collective_compute(
                "AllGather", mybir.AluOpType.bypass, replica_groups=RG,
                ins=[scb.ap().opt()], outs=[sc_full.ap().opt()],
            )

            # ================= M1 = scores^T @ W1 (h-shard) =================
            with (
                tc.tile_pool(name="m1pool", bufs=1) as m1pool,
                tc.tile_pool(name="m1psum", bufs=3, space="PSUM") as m1psum,
            ):
                sc_sb = m1pool.tile([P, IT, D], bf16)
                nc.sync.dma_start(
                    out=sc_sb[:],
                    in_=sc_full.ap().rearrange("(it p) j -> p it j", p=P),
                )
                for m in "qk":
                    w1_sb = m1pool.tile([P, IT, HS], bf16, tag=f"w1_{m}")
                    nc.sync.dma_start(
                        out=w1_sb[:],
                        in_=w1s[m].ap().rearrange("(it p) h -> p it h", p=P),
                    )
                    for half in range(2):
                        for jj in range(IT // 2):
                            jm = half * (IT // 2) + jj
                            mp = m1psum.tile([P, HS], f32, tag="m1ps",
                                             name=f"mp_{m}{jm}")
                            for it in range(IT):
                                nc.tensor.matmul(
                                    mp[:],
                                    sc_sb[:, it, jm * P:(jm + 1) * P],
                                    w1_sb[:, it, :],
                                    start=(it == 0),
                                    stop=(it == IT - 1),
                                )
                            mo = doutp.tile([P, HS], bf16, tag="m1out",
                                            name=f"mo_{m}{jm}")
                            nc.vector.tensor_copy(out=mo[:], in_=mp[:])
                            nc.sync.dma_start(
                                out=m1s[m, half].ap()[jj * P:(jj + 1) * P, :],
                                in_=mo[:],
                            )
                        nc.gpsimd.collective_compute(
                            "AllGather", mybir.AluOpType.bypass, replica_groups=RG,
                            ins=[m1s[m, half].ap().opt()],
                            outs=[m1f[m, half].ap().opt()],
                        )

            # ================= MLPs =================
            with (
                tc.tile_pool(name="mlp", bufs=1) as mlp,
                tc.tile_pool(name="vstream", bufs=2) as vstream,
                tc.tile_pool(name="mlppsum", bufs=5, space="PSUM") as bpsum,
                tc.tile_pool(name="cpsum", bufs=3, space="PSUM") as cpsum,
            ):
                for m in "qk":
                    m1_half = []
                    for half in range(2):
                        mh = mlp.tile([P, IT // 2, H], bf16, tag=f"m1big{half}",
                                      name=f"m1big{half}_{m}")
                        for c2 in range(NCORES):
                            nc.sync.dma_start(
                                out=mh[:, :, c2 * HS:(c2 + 1) * HS],
                                in_=m1f[m, half].ap()[c2]
                                    .rearrange("(jb p) h -> p jb h", p=P),
                            )
                        m1_half.append(mh)
                    w2_sb = mlp.tile([P, HB, D], bf16, tag="w2big")
                    nc.sync.dma_start(
                        out=w2_sb[:],
                        in_=w2[m].ap().rearrange("(hb p) d -> p hb d", p=P),
                    )
                    b1_sb = small.tile([P, H // P], f32, tag="b1t")
                    nc.sync.dma_start(out=b1_sb[:], in_=b1t[m].ap())
                    b2_sb = small.tile([P, D], f32, tag="b2r")
                    b2_bcast = b2r[m].ap()
                    nc.sync.dma_start(
                        out=b2_sb[:],
                        in_=bass.AP(tensor=b2_bcast.tensor, offset=b2_bcast.offset,
                                    ap=[[0, P], *b2_bcast.ap[1:]]),
                    )

                    for ncnk in range(NS // JW):      # 8 chunks of 512 samples
                        vt_sb = vstream.tile([P, IT, JW], bf16, tag="vt")
                        nc.sync.dma_start(
                            out=vt_sb[:],
                            in_=vt.ap()[:, ncnk * JW:(ncnk + 1) * JW]
                                .rearrange("(jb p) n -> p jb n", p=P),
                        )
                        hid_sb = mlp.tile([P, HB, JW], bf16, tag="hid")
                        # hiddenT[h, n] = relu(sum_j M1[j,h] vT[j,n] + b1[h])
                        for hb in range(HB):
                            pb = bpsum.tile([P, JW], f32, tag="psB")
                            for jb in range(IT):
                                nc.tensor.matmul(
                                    pb[:],
                                    m1_half[jb // (IT // 2)][:, jb % (IT // 2),
                                                             hb * P:(hb + 1) * P],
                                    vt_sb[:, jb, :],
                                    start=(jb == 0),
                                    stop=(jb == IT - 1),
                                )
                            nc.vector.tensor_scalar(
                                out=hid_sb[:, hb, :], in0=pb[:],
                                scalar1=b1_sb[:, hb:hb + 1], scalar2=0.0,
                                op0=add, op1=mx,
                            )
                        # dom[n, i2] = sum_h hidden[n,h] W2[h,i2] + b2[i2]
                        for ns in range(JW // P):     # 4 sample sub-tiles
                            for ih in range(JH):      # 2 output column halves
                                pc = cpsum.tile([P, JW], f32, tag="psC")
                                for hb in range(HB):
                                    nc.tensor.matmul(
                                        pc[:],
                                        hid_sb[:, hb, ns * P:(ns + 1) * P],
                                        w2_sb[:, hb, ih * JW:(ih + 1) * JW],
                                        start=(hb == 0), stop=(hb == HB - 1),
                                    )
                                do = doutp.tile([P, JW], f32, tag="dmout")
                                nc.vector.tensor_tensor(
                                    out=do[:], in0=pc[:],
                                    in1=b2_sb[:, ih * JW:(ih + 1) * JW],
                                    op=add,
                                )
                                nc.sync.dma_start(
                                    out=dom[m].ap()[
                                        ncnk * JW + ns * P:ncnk * JW + (ns + 1) * P,
                                        ih * JW:(ih + 1) * JW],
                                    in_=do[:],
                                )

    nc.compile()
    return nc


def _get_nc():
    if "nc" not in _CACHE:
        _CACHE["nc"] = _build()
    return _CACHE["nc"]


def _make_in_maps(inputs):
    query = np.asarray(inputs["query"])
    key = np.asarray(inputs["key"])
    value = np.asarray(inputs["value"])

    q_bf = query.astype(BF)
    k_bf = key.astype(BF)
    vt_bf = np.ascontiguousarray(value.T).astype(BF)          # [D, N]
    w1 = {"q": inputs["wq1"], "k": inputs["wk1"]}
    w2 = {"q": inputs["wq2"], "k": inputs["wk2"]}
    b1 = {"q": inputs["bq1"], "k": inputs["bk1"]}
    b2 = {"q": inputs["bq2"], "k": inputs["bk2"]}
    w1_bf = {m: np.asarray(w1[m]).astype(BF) for m in "qk"}
    w2_bf = {m: np.ascontiguousarray(np.asarray(w2[m]).astype(BF)) for m in "qk"}
    b1_t = {m: np.ascontiguousarray(
        np.asarray(b1[m]).astype(np.float32).reshape(H // P, P).T) for m in "qk"}
    b2_r = {m: np.asarray(b2[m]).astype(np.float32).reshape(1, D) for m in "qk"}

    in_maps = []
    diag = 1.0 - 1.0 / np.sqrt(D).astype(np.float32)
    for c in range(NCORES):
        msk = np.ones((P, D), np.float32)
        msk[np.arange(P), c * P + np.arange(P)] = diag
        im = {
            "q": np.ascontiguousarray(q_bf[c * NS:(c + 1) * NS]),
            "k": np.ascontiguousarray(k_bf[c * NS:(c + 1) * NS]),
            "vt": np.ascontiguousarray(vt_bf[:, c * NS:(c + 1) * NS]),
            "mask": msk,
        }
        for m in "qk":
            im[f"w1s_{m}"] = np.ascontiguousarray(
                w1_bf[m][:, c * HS:(c + 1) * HS])
            im[f"w2_{m}"] = w2_bf[m]
            im[f"b1t_{m}"] = b1_t[m]
            im[f"b2r_{m}"] = b2_r[m]
        in_maps.append(im)
    return in_maps


def _gather(results):
    dom_q = np.concatenate([results[c]["dom_q"] for c in range(NCORES)], axis=0)
    dom_k = np.concatenate([results[c]["dom_k"] for c in range(NCORES)], axis=0)
    return dom_q, dom_k


def _run(inputs, **kw):
    from concourse import bass_utils
    nc = _get_nc()
    in_maps = _make_in_maps(inputs)
    return bass_utils.run_bass_kernel_spmd(
        nc, in_maps, core_ids=list(range(NCORES)), **kw
    )


def kernel(**inputs):
    res = _run(inputs)
    return _gather(res.results)


# revision 13
# speedup vs baseline: 1.0168x; 1.0140x over previous
"""Trainium2 Bass kernel for nn_DomainAdaptation (sparse feature-attention + dual MLP).

Math (reference):
    S = Q^T K                        [D, D], contraction over N
    L = exp(S - S*I/sqrt(D))
    scores = softmax(L, axis=-1)
    attn = (scores @ V^T)^T          [N, D]
    dom_q = relu(attn @ Wq1 + bq1) @ Wq2 + bq2
    dom_k = relu(attn @ Wk1 + bk1) @ Wk2 + bk2

Key restructuring: attn = V @ scores^T, so
    attn @ W1 = V @ (scores^T @ W1) = V @ M1
and attn is never materialized. Per core (N sharded 8 ways):
    phase 1: S_partial = Qc^T Kc  (bf16 matmuls, f32 accum)
    ReduceScatter(S) -> each core owns a 128-row block of S
    softmax block (f32, incl. double-exp of the reference), AllGather(scores)
    M1 = scores^T @ W1, h-sharded per core, AllGather(M1)  [x2 for q/k]
    MLP: hiddenT = relu(M1^T @ Vc^T + b1);  dom = hidden^T-contracted @ W2 + b2
outputs in natural [N, D] f32 orientation.
"""

import numpy as np
import ml_dtypes

N, D, H = 32768, 1024, 4096
NCORES = 8
NS = N // NCORES          # 4096 sample rows per core
HS = H // NCORES          # 512 hidden cols per core (M1 shard)
P = 128
BF = ml_dtypes.bfloat16

_CACHE: dict = {}


def _build():
    import concourse.bass as bass
    import concourse.tile as tile
    from concourse import bacc, mybir

    f32 = mybir.dt.float32
    bf16 = mybir.dt.bfloat16
    Exp = mybir.ActivationFunctionType.Exp
    add = mybir.AluOpType.add
    mx = mybir.AluOpType.max
    mult = mybir.AluOpType.mult

    nc = bacc.Bacc("TRN2", target_bir_lowering=False, debug=False, num_devices=NCORES)

    # ---- I/O ----
    q = nc.dram_tensor("q", [NS, D], bf16, kind="ExternalInput")
    k = nc.dram_tensor("k", [NS, D], bf16, kind="ExternalInput")
    vt = nc.dram_tensor("vt", [D, NS], bf16, kind="ExternalInput")
    w1s = {m: nc.dram_tensor(f"w1s_{m}", [D, HS], bf16, kind="ExternalInput") for m in "qk"}
    w2 = {m: nc.dram_tensor(f"w2_{m}", [H, D], bf16, kind="ExternalInput") for m in "qk"}
    b1t = {m: nc.dram_tensor(f"b1t_{m}", [P, H // P], f32, kind="ExternalInput") for m in "qk"}
    b2r = {m: nc.dram_tensor(f"b2r_{m}", [1, D], f32, kind="ExternalInput") for m in "qk"}
    mask = nc.dram_tensor("mask", [P, D], f32, kind="ExternalInput")
    dom = {m: nc.dram_tensor(f"dom_{m}", [NS, D], f32, kind="ExternalOutput") for m in "qk"}

    # ---- internal DRAM (collective bounce buffers) ----
    s_part = [nc.dram_tensor(f"s_part{j}", [D, 512], f32) for j in range(2)]
    s_red = [nc.dram_tensor(f"s_red{j}", [P, 512], f32) for j in range(2)]
    scb = nc.dram_tensor("scb", [P, D], bf16)
    sc_full = nc.dram_tensor("sc_full", [D, D], bf16, addr_space="Shared")
    m1s = {(m, h): nc.dram_tensor(f"m1s_{m}{h}", [D // 2, HS], bf16)
           for m in "qk" for h in range(2)}
    m1f = {(m, h): nc.dram_tensor(f"m1f_{m}{h}", [NCORES, D // 2, HS], bf16,
                                  addr_space="Shared")
           for m in "qk" for h in range(2)}

    RG = [list(range(NCORES))]
    NB = NS // P              # 32 n-blocks per core
    IT = D // P               # 8 feature tiles
    JW = 512                  # matmul moving free dim
    JH = D // JW              # 2 j-halves of S
    HB = H // P               # 32 hidden blocks
    KO = 4                    # phase-1 k-stream chunks (of NB//KO n-blocks each)
    NBC = NB // KO            # 8 n-blocks per stream chunk

    with tile.TileContext(nc) as tc:
        with (
            tc.tile_pool(name="small", bufs=1) as small,
            tc.tile_pool(name="dout", bufs=4) as doutp,
            tc.tile_pool(name="wpool", bufs=1) as wpool,
        ):
            mask_sb = small.tile([P, D], f32)
            nc.sync.dma_start(out=mask_sb[:], in_=mask.ap())

            w2_tiles = {}
            w2_tiles["q"] = wpool.tile([P, HB, D], bf16, tag="w2big", name="w2_q")
            nc.sync.dma_start(
                out=w2_tiles["q"][:],
                in_=w2["q"].ap().rearrange("(hb p) d -> p hb d", p=P),
            )

            # ================= phase 1: S_partial = Qc^T Kc =================
            with (
                tc.tile_pool(name="ph1", bufs=1) as ph1,
                tc.tile_pool(name="kstream", bufs=2) as kstream,
                tc.tile_pool(name="ph1psum", bufs=1, space="PSUM") as ph1psum,
            ):
                q_ch = {}
                for jh in range(JH):
                    ps = [
                        ph1psum.tile([P, JW], f32, tag=f"sps{i}", name=f"sps{i}_{jh}")
                        for i in range(IT)
                    ]
                    for ko in range(KO):
                        if ko not in q_ch:
                            qc = ph1.tile([P, NBC, D], bf16, tag=f"qc{ko}",
                                          name=f"qc{ko}")
                            nc.sync.dma_start(
                                out=qc[:],
                                in_=q.ap()[ko * NBC * P:(ko + 1) * NBC * P, :]
                                    .rearrange("(nb p) d -> p nb d", p=P),
                            )
                            q_ch[ko] = qc
                        k_sb = kstream.tile([P, NBC, JW], bf16, tag="kc")
                        nc.sync.dma_start(
                            out=k_sb[:],
                            in_=k.ap()[ko * NBC * P:(ko + 1) * NBC * P,
                                       jh * JW:(jh + 1) * JW]
                                .rearrange("(nb p) d -> p nb d", p=P),
                        )
                        for nb in range(NBC):
                            for i in range(IT):
                                nc.tensor.matmul(
                                    ps[i][:],
                                    q_ch[ko][:, nb, i * P:(i + 1) * P],
                                    k_sb[:, nb, :],
                                    start=(ko == 0 and nb == 0),
                                    stop=(ko == KO - 1 and nb == NBC - 1),
                                )
                    for i in range(IT):
                        so = doutp.tile([P, JW], f32, tag="sout")
                        nc.vector.tensor_copy(out=so[:], in_=ps[i][:])
                        nc.sync.dma_start(
                            out=s_part[jh].ap()[i * P:(i + 1) * P, :],
                            in_=so[:],
                        )
                    # ReduceScatter this column-half; the jh=0 one overlaps
                    # the jh=1 matmuls.
                    nc.gpsimd.collective_compute(
                        "ReduceScatter", add, replica_groups=RG,
                        ins=[s_part[jh].ap().opt()], outs=[s_red[jh].ap().opt()],
                    )

            # ================= softmax block (f32) =================
            with tc.tile_pool(name="smx", bufs=1) as smx:
                sred_sb = smx.tile([P, D], f32)
                for j in range(2):
                    nc.sync.dma_start(out=sred_sb[:, j * JW:(j + 1) * JW],
                                      in_=s_red[j].ap())
                tmask = smx.tile([P, D], f32)
                nc.vector.tensor_tensor(out=tmask[:], in0=sred_sb[:], in1=mask_sb[:],
                                        op=mult)
                logit = smx.tile([P, D], f32)
                nc.scalar.activation(out=logit[:], in_=tmask[:], func=Exp)
                e2 = smx.tile([P, D], f32)
                zsum = smx.tile([P, 1], f32)
                nc.scalar.activation(out=e2[:], in_=logit[:], func=Exp,
                                     accum_out=zsum[:])
                rz = smx.tile([P, 1], f32)
                nc.vector.reciprocal(rz[:], zsum[:])
                scb_sb = smx.tile([P, D], bf16)
                nc.vector.tensor_scalar(out=scb_sb[:], in0=e2[:], scalar1=rz[:],
                                        scalar2=None, op0=mult)
                nc.sync.dma_start(out=scb.ap(), in_=scb_sb[:])

            nc.gpsimd.collective_compute(
                "AllGather", mybir.AluOpType.bypass, replica_groups=RG,
                ins=[scb.ap().opt()], outs=[sc_full.ap().opt()],
            )

            # ================= M1 = scores^T @ W1 (h-shard) =================
            with (
                tc.tile_pool(name="m1pool", bufs=1) as m1pool,
                tc.tile_pool(name="m1psum", bufs=3, space="PSUM") as m1psum,
            ):
                sc_sb = m1pool.tile([P, IT, D], bf16)
                nc.sync.dma_start(
                    out=sc_sb[:],
                    in_=sc_full.ap().rearrange("(it p) j -> p it j", p=P),
                )
                for m in "qk":
                    w1_sb = m1pool.tile([P, IT, HS], bf16, tag=f"w1_{m}")
                    nc.sync.dma_start(
                        out=w1_sb[:],
                        in_=w1s[m].ap().rearrange("(it p) h -> p it h", p=P),
                    )
                    for half in range(2):
                        for jj in range(IT // 2):
                            jm = half * (IT // 2) + jj
                            mp = m1psum.tile([P, HS], f32, tag="m1ps",
                                             name=f"mp_{m}{jm}")
                            for it in range(IT):
                                nc.tensor.matmul(
                                    mp[:],
                                    sc_sb[:, it, jm * P:(jm + 1) * P],
                                    w1_sb[:, it, :],
                                    start=(it == 0),
                                    stop=(it == IT - 1),
                                )
                            mo = doutp.tile([P, HS], bf16, tag="m1out",
                                            name=f"mo_{m}{jm}")
                            nc.vector.tensor_copy(out=mo[:], in_=mp[:])
                            nc.sync.dma_start(
                                out=m1s[m, half].ap()[jj * P:(jj + 1) * P, :],
                                in_=mo[:],
                            )
                        nc.gpsimd.collective_compute(
                            "AllGather", mybir.AluOpType.bypass, replica_groups=RG,
                            ins=[m1s[m, half].ap().opt()],
                            outs=[m1f[m, half].ap().opt()],
                        )

            # ================= MLPs =================
            with (
                tc.tile_pool(name="mlp", bufs=1) as mlp,
                tc.tile_pool(name="vstream", bufs=2) as vstream,
                tc.tile_pool(name="mlppsum", bufs=5, space="PSUM") as bpsum,
                tc.tile_pool(name="cpsum", bufs=3, space="PSUM") as cpsum,
            ):
                for m in "qk":
                    m1_half = []
                    for half in range(2):
                        mh = mlp.tile([P, IT // 2, H], bf16, tag=f"m1big{half}",
                                      name=f"m1big{half}_{m}")
                        for c2 in range(NCORES):
                            nc.sync.dma_start(
                                out=mh[:, :, c2 * HS:(c2 + 1) * HS],
                                in_=m1f[m, half].ap()[c2]
                                    .rearrange("(jb p) h -> p jb h", p=P),
                            )
                        m1_half.append(mh)
                    if m in w2_tiles:
                        w2_sb = w2_tiles[m]
                    else:
                        w2_sb = wpool.tile([P, HB, D], bf16, tag="w2big",
                                           name=f"w2_{m}")
                        nc.sync.dma_start(
                            out=w2_sb[:],
                            in_=w2[m].ap().rearrange("(hb p) d -> p hb d", p=P),
                        )
                    b1_sb = small.tile([P, H // P], f32, tag="b1t")
                    nc.sync.dma_start(out=b1_sb[:], in_=b1t[m].ap())
                    b2_sb = small.tile([P, D], f32, tag="b2r")
                    b2_bcast = b2r[m].ap()
                    nc.sync.dma_start(
                        out=b2_sb[:],
                        in_=bass.AP(tensor=b2_bcast.tensor, offset=b2_bcast.offset,
                                    ap=[[0, P], *b2_bcast.ap[1:]]),
                    )

                    for ncnk in range(NS // JW):      # 8 chunks of 512 samples
                        vt_sb = vstream.tile([P, IT, JW], bf16, tag="vt")
                        nc.sync.dma_start(
                            out=vt_sb[:],
                            in_=vt.ap()[:, ncnk * JW:(ncnk + 1) * JW]
                                .rearrange("(jb p) n -> p jb n", p=P),
                        )
                        hid_sb = mlp.tile([P, HB, JW], bf16, tag="hid")
                        # hiddenT[h, n] = relu(sum_j M1[j,h] vT[j,n] + b1[h])
                        for hb in range(HB):
                            pb = bpsum.tile([P, JW], f32, tag="psB")
                            for jb in range(IT):
                                nc.tensor.matmul(
                                    pb[:],
                                    m1_half[jb // (IT // 2)][:, jb % (IT // 2),
                                                             hb * P:(hb + 1) * P],
                                    vt_sb[:, jb, :],
                                    start=(jb == 0),
                                    stop=(jb == IT - 1),
                                )
                            nc.vector.tensor_scalar(
                                out=hid_sb[:, hb, :], in0=pb[:],
                                scalar1=b1_sb[:, hb:hb + 1], scalar2=0.0,
                                op0=add, op1=mx,
                            )
                        # dom[n, i2] = sum_h hidden[n,h] W2[h,i2] + b2[i2]
                        for ns in range(JW // P):     # 4 sample sub-tiles
                            for ih in range(JH):      # 2 output column halves
                                pc = cpsum.tile([P, JW], f32, tag="psC")
                                for hb in range(HB):
                                    nc.tensor.matmul(
                                        pc[:],
                                        hid_sb[:, hb, ns * P:(ns + 1) * P],
                                        w2_sb[:, hb, ih * JW:(ih + 1) * JW],
                                        start=(hb == 0), stop=(hb == HB - 1),
                                    )
                                do = doutp.tile([P, JW], f32, tag="dmout")
                                nc.vector.tensor_tensor(
                                    out=do[:], in0=pc[:],
                                    in1=b2_sb[:, ih * JW:(ih + 1) * JW],
                                    op=add,
                                )
                                nc.sync.dma_start(
                                    out=dom[m].ap()[
                                        ncnk * JW + ns * P:ncnk * JW + (ns + 1) * P,
                                        ih * JW:(ih + 1) * JW],
                                    in_=do[:],
                                )

    nc.compile()
    return nc


def _get_nc():
    if "nc" not in _CACHE:
        _CACHE["nc"] = _build()
    return _CACHE["nc"]


def _make_in_maps(inputs):
    query = np.asarray(inputs["query"])
    key = np.asarray(inputs["key"])
    value = np.asarray(inputs["value"])

    q_bf = query.astype(BF)
    k_bf = key.astype(BF)
    vt_bf = np.ascontiguousarray(value.T).astype(BF)          # [D, N]
    w1 = {"q": inputs["wq1"], "k": inputs["wk1"]}
    w2 = {"q": inputs["wq2"], "k": inputs["wk2"]}
    b1 = {"q": inputs["bq1"], "k": inputs["bk1"]}
    b2 = {"q": inputs["bq2"], "k": inputs["bk2"]}
    w1_bf = {m: np.asarray(w1[m]).astype(BF) for m in "qk"}
    w2_bf = {m: np.ascontiguousarray(np.asarray(w2[m]).astype(BF)) for m in "qk"}
    b1_t = {m: np.ascontiguousarray(
        np.asarray(b1[m]).astype(np.float32).reshape(H // P, P).T) for m in "qk"}
    b2_r = {m: np.asarray(b2[m]).astype(np.float32).reshape(1, D) for m in "qk"}

    in_maps = []
    diag = 1.0 - 1.0 / np.sqrt(D).astype(np.float32)
    for c in range(NCORES):
        msk = np.ones((P, D), np.float32)
        msk[np.arange(P), c * P + np.arange(P)] = diag
        im = {
            "q": np.ascontiguousarray(q_bf[c * NS:(c + 1) * NS]),
            "k": np.ascontiguousarray(k_bf[c * NS:(c + 1) * NS]),
            "vt": np.ascontiguousarray(vt_bf[:, c * NS:(c + 1) * NS]),
            "mask": msk,
        }
        for m in "qk":
            im[f"w1s_{m}"] = np.ascontiguousarray(
                w1_bf[m][:, c * HS:(c + 1) * HS])
            im[f"w2_{m}"] = w2_bf[m]
            im[f"b1t_{m}"] = b1_t[m]
            im[f"b2r_{m}"] = b2_r[m]
        in_maps.append(im)
    return in_maps


def _gather(results):
    dom_q = np.concatenate([results[c]["dom_q"] for c in range(NCORES)], axis=0)
    dom_k = np.concatenate([results[c]["dom_k"] for c in range(NCORES)], axis=0)
    return dom_q, dom_k


def _run(inputs, **kw):
    from concourse import bass_utils
    nc = _get_nc()
    in_maps = _make_in_maps(inputs)
    return bass_utils.run_bass_kernel_spmd(
        nc, in_maps, core_ids=list(range(NCORES)), **kw
    )


def kernel(**inputs):
    res = _run(inputs)
    return _gather(res.results)


# revision 14
# speedup vs baseline: 1.0316x; 1.0146x over previous
"""Trainium2 Bass kernel for nn_DomainAdaptation (sparse feature-attention + dual MLP).

Math (reference):
    S = Q^T K                        [D, D], contraction over N
    L = exp(S - S*I/sqrt(D))
    scores = softmax(L, axis=-1)
    attn = (scores @ V^T)^T          [N, D]
    dom_q = relu(attn @ Wq1 + bq1) @ Wq2 + bq2
    dom_k = relu(attn @ Wk1 + bk1) @ Wk2 + bk2

Key restructuring: attn = V @ scores^T, so
    attn @ W1 = V @ (scores^T @ W1) = V @ M1
and attn is never materialized. Per core (N sharded 8 ways):
    phase 1: S_partial = Qc^T Kc  (bf16 matmuls, f32 accum)
    ReduceScatter(S) -> each core owns a 128-row block of S
    softmax block (f32, incl. double-exp of the reference), AllGather(scores)
    M1 = scores^T @ W1, h-sharded per core, AllGather(M1)  [x2 for q/k]
    MLP: hiddenT = relu(M1^T @ Vc^T + b1);  dom = hidden^T-contracted @ W2 + b2
outputs in natural [N, D] f32 orientation.
"""

import numpy as np
import ml_dtypes

N, D, H = 32768, 1024, 4096
NCORES = 8
NS = N // NCORES          # 4096 sample rows per core
HS = H // NCORES          # 512 hidden cols per core (M1 shard)
P = 128
BF = ml_dtypes.bfloat16

_CACHE: dict = {}


def _build():
    import concourse.bass as bass
    import concourse.tile as tile
    from concourse import bacc, mybir

    f32 = mybir.dt.float32
    bf16 = mybir.dt.bfloat16
    Exp = mybir.ActivationFunctionType.Exp
    add = mybir.AluOpType.add
    mx = mybir.AluOpType.max
    mult = mybir.AluOpType.mult

    nc = bacc.Bacc("TRN2", target_bir_lowering=False, debug=False, num_devices=NCORES)

    # ---- I/O ----
    q = nc.dram_tensor("q", [NS, D], bf16, kind="ExternalInput")
    k = nc.dram_tensor("k", [NS, D], bf16, kind="ExternalInput")
    vt = nc.dram_tensor("vt", [D, NS], bf16, kind="ExternalInput")
    w1s = {m: nc.dram_tensor(f"w1s_{m}", [D, HS], bf16, kind="ExternalInput") for m in "qk"}
    w2 = {m: nc.dram_tensor(f"w2_{m}", [H, D], bf16, kind="ExternalInput") for m in "qk"}
    b1t = {m: nc.dram_tensor(f"b1t_{m}", [P, H // P], f32, kind="ExternalInput") for m in "qk"}
    b2r = {m: nc.dram_tensor(f"b2r_{m}", [1, D], f32, kind="ExternalInput") for m in "qk"}
    mask = nc.dram_tensor("mask", [P, D], f32, kind="ExternalInput")
    dom = {m: nc.dram_tensor(f"dom_{m}", [NS, D], f32, kind="ExternalOutput") for m in "qk"}

    # ---- internal DRAM (collective bounce buffers) ----
    s_part = [nc.dram_tensor(f"s_part{j}", [D, 512], f32) for j in range(2)]
    s_red = [nc.dram_tensor(f"s_red{j}", [P, 512], f32) for j in range(2)]
    scb = nc.dram_tensor("scb", [P, D], bf16)
    sc_full = nc.dram_tensor("sc_full", [D, D], bf16, addr_space="Shared")
    m1s = {(m, h): nc.dram_tensor(f"m1s_{m}{h}", [D, HS // 2], bf16)
           for m in "qk" for h in range(2)}
    m1f = {(m, h): nc.dram_tensor(f"m1f_{m}{h}", [NCORES, D, HS // 2], bf16,
                                  addr_space="Shared")
           for m in "qk" for h in range(2)}

    RG = [list(range(NCORES))]
    NB = NS // P              # 32 n-blocks per core
    IT = D // P               # 8 feature tiles
    JW = 512                  # matmul moving free dim
    JH = D // JW              # 2 j-halves of S
    HB = H // P               # 32 hidden blocks
    KO = 4                    # phase-1 k-stream chunks (of NB//KO n-blocks each)
    NBC = NB // KO            # 8 n-blocks per stream chunk

    with tile.TileContext(nc) as tc:
        with (
            tc.tile_pool(name="small", bufs=1) as small,
            tc.tile_pool(name="dout", bufs=4) as doutp,
            tc.tile_pool(name="wpool", bufs=1) as wpool,
        ):
            mask_sb = small.tile([P, D], f32)
            w2_tiles = {}
            w2_tiles["q"] = wpool.tile([P, HB, D], bf16, tag="w2big", name="w2_q")

            # ================= phase 1: S_partial = Qc^T Kc =================
            with (
                tc.tile_pool(name="ph1", bufs=1) as ph1,
                tc.tile_pool(name="kstream", bufs=2) as kstream,
                tc.tile_pool(name="ph1psum", bufs=1, space="PSUM") as ph1psum,
            ):
                q_ch = {}
                for jh in range(JH):
                    ps = [
                        ph1psum.tile([P, JW], f32, tag=f"sps{i}", name=f"sps{i}_{jh}")
                        for i in range(IT)
                    ]
                    for ko in range(KO):
                        if ko not in q_ch:
                            qc = ph1.tile([P, NBC, D], bf16, tag=f"qc{ko}",
                                          name=f"qc{ko}")
                            nc.sync.dma_start(
                                out=qc[:],
                                in_=q.ap()[ko * NBC * P:(ko + 1) * NBC * P, :]
                                    .rearrange("(nb p) d -> p nb d", p=P),
                            )
                            q_ch[ko] = qc
                        k_sb = kstream.tile([P, NBC, JW], bf16, tag="kc")
                        nc.sync.dma_start(
                            out=k_sb[:],
                            in_=k.ap()[ko * NBC * P:(ko + 1) * NBC * P,
                                       jh * JW:(jh + 1) * JW]
                                .rearrange("(nb p) d -> p nb d", p=P),
                        )
                        # trickle-load mask + next MLP's w2 behind the
                        # phase-1 operand stream
                        idx = jh * KO + ko
                        if idx == 0:
                            nc.sync.dma_start(out=mask_sb[:], in_=mask.ap())
                        nc.sync.dma_start(
                            out=w2_tiles["q"][:, idx * (HB // 8):(idx + 1) * (HB // 8), :],
                            in_=w2["q"].ap()
                                .rearrange("(hb p) d -> p hb d", p=P)[
                                    :, idx * (HB // 8):(idx + 1) * (HB // 8), :],
                        )
                        for nb in range(NBC):
                            for i in range(IT):
                                nc.tensor.matmul(
                                    ps[i][:],
                                    q_ch[ko][:, nb, i * P:(i + 1) * P],
                                    k_sb[:, nb, :],
                                    start=(ko == 0 and nb == 0),
                                    stop=(ko == KO - 1 and nb == NBC - 1),
                                )
                    for i in range(IT):
                        so = doutp.tile([P, JW], f32, tag="sout")
                        nc.vector.tensor_copy(out=so[:], in_=ps[i][:])
                        nc.sync.dma_start(
                            out=s_part[jh].ap()[i * P:(i + 1) * P, :],
                            in_=so[:],
                        )
                    # ReduceScatter this column-half; the jh=0 one overlaps
                    # the jh=1 matmuls.
                    nc.gpsimd.collective_compute(
                        "ReduceScatter", add, replica_groups=RG,
                        ins=[s_part[jh].ap().opt()], outs=[s_red[jh].ap().opt()],
                    )

            # ================= softmax block (f32) =================
            with tc.tile_pool(name="smx", bufs=1) as smx:
                sred_sb = smx.tile([P, D], f32)
                for j in range(2):
                    nc.sync.dma_start(out=sred_sb[:, j * JW:(j + 1) * JW],
                                      in_=s_red[j].ap())
                tmask = smx.tile([P, D], f32)
                nc.vector.tensor_tensor(out=tmask[:], in0=sred_sb[:], in1=mask_sb[:],
                                        op=mult)
                logit = smx.tile([P, D], f32)
                nc.scalar.activation(out=logit[:], in_=tmask[:], func=Exp)
                e2 = smx.tile([P, D], f32)
                zsum = smx.tile([P, 1], f32)
                nc.scalar.activation(out=e2[:], in_=logit[:], func=Exp,
                                     accum_out=zsum[:])
                rz = smx.tile([P, 1], f32)
                nc.vector.reciprocal(rz[:], zsum[:])
                scb_sb = smx.tile([P, D], bf16)
                nc.vector.tensor_scalar(out=scb_sb[:], in0=e2[:], scalar1=rz[:],
                                        scalar2=None, op0=mult)
                nc.sync.dma_start(out=scb.ap(), in_=scb_sb[:])

            nc.gpsimd.collective_compute(
                "AllGather", mybir.AluOpType.bypass, replica_groups=RG,
                ins=[scb.ap().opt()], outs=[sc_full.ap().opt()],
            )

            # ================= M1 = scores^T @ W1 (h-shard) =================
            with (
                tc.tile_pool(name="m1pool", bufs=1) as m1pool,
                tc.tile_pool(name="m1psum", bufs=3, space="PSUM") as m1psum,
            ):
                sc_sb = m1pool.tile([P, IT, D], bf16)
                nc.sync.dma_start(
                    out=sc_sb[:],
                    in_=sc_full.ap().rearrange("(it p) j -> p it j", p=P),
                )
                for m in "qk":
                    w1_sb = m1pool.tile([P, IT, HS], bf16, tag=f"w1_{m}")
                    nc.sync.dma_start(
                        out=w1_sb[:],
                        in_=w1s[m].ap().rearrange("(it p) h -> p it h", p=P),
                    )
                    for jm in range(IT):
                        mp = m1psum.tile([P, HS], f32, tag="m1ps",
                                         name=f"mp_{m}{jm}")
                        for it in range(IT):
                            nc.tensor.matmul(
                                mp[:],
                                sc_sb[:, it, jm * P:(jm + 1) * P],
                                w1_sb[:, it, :],
                                start=(it == 0),
                                stop=(it == IT - 1),
                            )
                        mo = doutp.tile([P, HS], bf16, tag="m1out",
                                        name=f"mo_{m}{jm}")
                        nc.vector.tensor_copy(out=mo[:], in_=mp[:])
                        for half in range(2):
                            nc.sync.dma_start(
                                out=m1s[m, half].ap()[jm * P:(jm + 1) * P, :],
                                in_=mo[:, half * (HS // 2):(half + 1) * (HS // 2)],
                            )
                    for half in range(2):
                        nc.gpsimd.collective_compute(
                            "AllGather", mybir.AluOpType.bypass, replica_groups=RG,
                            ins=[m1s[m, half].ap().opt()],
                            outs=[m1f[m, half].ap().opt()],
                        )

            # ================= MLPs =================
            with (
                tc.tile_pool(name="mlp", bufs=1) as mlp,
                tc.tile_pool(name="vstream", bufs=2) as vstream,
                tc.tile_pool(name="mlppsum", bufs=5, space="PSUM") as bpsum,
                tc.tile_pool(name="cpsum", bufs=3, space="PSUM") as cpsum,
            ):
                for m in "qk":
                    HH = HS // 2
                    m1_half = []
                    for half in range(2):
                        mh = mlp.tile([P, IT, H // 2], bf16, tag=f"m1big{half}",
                                      name=f"m1big{half}_{m}")
                        for c2 in range(NCORES):
                            nc.sync.dma_start(
                                out=mh[:, :, c2 * HH:(c2 + 1) * HH],
                                in_=m1f[m, half].ap()[c2]
                                    .rearrange("(jb p) h -> p jb h", p=P),
                            )
                        m1_half.append(mh)
                    hb_order = [hb for hb in range(HB) if (hb % 4) < 2] + \
                               [hb for hb in range(HB) if (hb % 4) >= 2]
                    if m in w2_tiles:
                        w2_sb = w2_tiles[m]
                    else:
                        w2_sb = wpool.tile([P, HB, D], bf16, tag="w2big",
                                           name=f"w2_{m}")
                        nc.sync.dma_start(
                            out=w2_sb[:],
                            in_=w2[m].ap().rearrange("(hb p) d -> p hb d", p=P),
                        )
                    b1_sb = small.tile([P, H // P], f32, tag="b1t")
                    nc.sync.dma_start(out=b1_sb[:], in_=b1t[m].ap())
                    b2_sb = small.tile([P, D], f32, tag="b2r")
                    b2_bcast = b2r[m].ap()
                    nc.sync.dma_start(
                        out=b2_sb[:],
                        in_=bass.AP(tensor=b2_bcast.tensor, offset=b2_bcast.offset,
                                    ap=[[0, P], *b2_bcast.ap[1:]]),
                    )

                    for ncnk in range(NS // JW):      # 8 chunks of 512 samples
                        vt_sb = vstream.tile([P, IT, JW], bf16, tag="vt")
                        nc.sync.dma_start(
                            out=vt_sb[:],
                            in_=vt.ap()[:, ncnk * JW:(ncnk + 1) * JW]
                                .rearrange("(jb p) n -> p jb n", p=P),
                        )
                        hid_sb = mlp.tile([P, HB, JW], bf16, tag="hid")
                        # hiddenT[h, n] = relu(sum_j M1[j,h] vT[j,n] + b1[h])
                        for hb in hb_order:
                            c2, pos = hb // 4, hb % 4
                            half, hh = pos // 2, pos % 2
                            off = c2 * HH + hh * P
                            pb = bpsum.tile([P, JW], f32, tag="psB")
                            for jb in range(IT):
                                nc.tensor.matmul(
                                    pb[:],
                                    m1_half[half][:, jb, off:off + P],
                                    vt_sb[:, jb, :],
                                    start=(jb == 0),
                                    stop=(jb == IT - 1),
                                )
                            nc.vector.tensor_scalar(
                                out=hid_sb[:, hb, :], in0=pb[:],
                                scalar1=b1_sb[:, hb:hb + 1], scalar2=0.0,
                                op0=add, op1=mx,
                            )
                        # dom[n, i2] = sum_h hidden[n,h] W2[h,i2] + b2[i2]
                        for ns in range(JW // P):     # 4 sample sub-tiles
                            for ih in range(JH):      # 2 output column halves
                                pc = cpsum.tile([P, JW], f32, tag="psC")
                                for hb in range(HB):
                                    nc.tensor.matmul(
                                        pc[:],
                                        hid_sb[:, hb, ns * P:(ns + 1) * P],
                                        w2_sb[:, hb, ih * JW:(ih + 1) * JW],
                                        start=(hb == 0), stop=(hb == HB - 1),
                                    )
                                do = doutp.tile([P, JW], f32, tag="dmout")
                                nc.vector.tensor_tensor(
                                    out=do[:], in0=pc[:],
                                    in1=b2_sb[:, ih * JW:(ih + 1) * JW],
                                    op=add,
                                )
                                nc.sync.dma_start(
                                    out=dom[m].ap()[
                                        ncnk * JW + ns * P:ncnk * JW + (ns + 1) * P,
                                        ih * JW:(ih + 1) * JW],
                                    in_=do[:],
                                )

    nc.compile()
    return nc


def _get_nc():
    if "nc" not in _CACHE:
        _CACHE["nc"] = _build()
    return _CACHE["nc"]


def _make_in_maps(inputs):
    query = np.asarray(inputs["query"])
    key = np.asarray(inputs["key"])
    value = np.asarray(inputs["value"])

    q_bf = query.astype(BF)
    k_bf = key.astype(BF)
    vt_bf = np.ascontiguousarray(value.T).astype(BF)          # [D, N]
    w1 = {"q": inputs["wq1"], "k": inputs["wk1"]}
    w2 = {"q": inputs["wq2"], "k": inputs["wk2"]}
    b1 = {"q": inputs["bq1"], "k": inputs["bk1"]}
    b2 = {"q": inputs["bq2"], "k": inputs["bk2"]}
    w1_bf = {m: np.asarray(w1[m]).astype(BF) for m in "qk"}
    w2_bf = {m: np.ascontiguousarray(np.asarray(w2[m]).astype(BF)) for m in "qk"}
    b1_t = {m: np.ascontiguousarray(
        np.asarray(b1[m]).astype(np.float32).reshape(H // P, P).T) for m in "qk"}
    b2_r = {m: np.asarray(b2[m]).astype(np.float32).reshape(1, D) for m in "qk"}

    in_maps = []
    diag = 1.0 - 1.0 / np.sqrt(D).astype(np.float32)
    for c in range(NCORES):
        msk = np.ones((P, D), np.float32)
        msk[np.arange(P), c * P + np.arange(P)] = diag
        im = {
            "q": np.ascontiguousarray(q_bf[c * NS:(c + 1) * NS]),
            "k": np.ascontiguousarray(k_bf[c * NS:(c + 1) * NS]),
            "vt": np.ascontiguousarray(vt_bf[:, c * NS:(c + 1) * NS]),
            "mask": msk,
        }
        for m in "qk":
            im[f"w1s_{m}"] = np.ascontiguousarray(
                w1_bf[m][:, c * HS:(c + 1) * HS])
            im[f"w2_{m}"] = w2_bf[m]
            im[f"b1t_{m}"] = b1_t[m]
            im[f"b2r_{m}"] = b2_r[m]
        in_maps.append(im)
    return in_maps


def _gather(results):
    dom_q = np.concatenate([results[c]["dom_q"] for c in range(NCORES)], axis=0)
    dom_k = np.concatenate([results[c]["dom_k"] for c in range(NCORES)], axis=0)
    return dom_q, dom_k


def _run(inputs, **kw):
    from concourse import bass_utils
    nc = _get_nc()
    in_maps = _make_in_maps(inputs)
    return bass_utils.run_bass_kernel_spmd(
        nc, in_maps, core_ids=list(range(NCORES)), **kw
    )


def kernel(**inputs):
    res = _run(inputs)
    return _gather(res.results)


# revision 16
# speedup vs baseline: 1.0336x; 1.0019x over previous
"""Trainium2 Bass kernel for nn_DomainAdaptation (sparse feature-attention + dual MLP).

Math (reference):
    S = Q^T K                        [D, D], contraction over N
    L = exp(S - S*I/sqrt(D))
    scores = softmax(L, axis=-1)
    attn = (scores @ V^T)^T          [N, D]
    dom_q = relu(attn @ Wq1 + bq1) @ Wq2 + bq2
    dom_k = relu(attn @ Wk1 + bk1) @ Wk2 + bk2

Key restructuring: attn = V @ scores^T, so
    attn @ W1 = V @ (scores^T @ W1) = V @ M1
and attn is never materialized. Per core (N sharded 8 ways):
    phase 1: S_partial = Qc^T Kc  (bf16 matmuls, f32 accum)
    ReduceScatter(S) -> each core owns a 128-row block of S
    softmax block (f32, incl. double-exp of the reference), AllGather(scores)
    M1 = scores^T @ W1, h-sharded per core, AllGather(M1)  [x2 for q/k]
    MLP: hiddenT = relu(M1^T @ Vc^T + b1);  dom = hidden^T-contracted @ W2 + b2
outputs in natural [N, D] f32 orientation.
"""

import numpy as np
import ml_dtypes

N, D, H = 32768, 1024, 4096
NCORES = 8
NS = N // NCORES          # 4096 sample rows per core
HS = H // NCORES          # 512 hidden cols per core (M1 shard)
P = 128
BF = ml_dtypes.bfloat16

_CACHE: dict = {}


def _build():
    import concourse.bass as bass
    import concourse.tile as tile
    from concourse import bacc, mybir

    f32 = mybir.dt.float32
    bf16 = mybir.dt.bfloat16
    Exp = mybir.ActivationFunctionType.Exp
    add = mybir.AluOpType.add
    mx = mybir.AluOpType.max
    mult = mybir.AluOpType.mult

    nc = bacc.Bacc("TRN2", target_bir_lowering=False, debug=False, num_devices=NCORES)

    # ---- I/O ----
    q = nc.dram_tensor("q", [NS, D], bf16, kind="ExternalInput")
    k = nc.dram_tensor("k", [NS, D], bf16, kind="ExternalInput")
    vt = nc.dram_tensor("vt", [D, NS], bf16, kind="ExternalInput")
    w1s = {m: nc.dram_tensor(f"w1s_{m}", [D, HS], bf16, kind="ExternalInput") for m in "qk"}
    w2 = {m: nc.dram_tensor(f"w2_{m}", [H, D], bf16, kind="ExternalInput") for m in "qk"}
    b1t = {m: nc.dram_tensor(f"b1t_{m}", [P, H // P], f32, kind="ExternalInput") for m in "qk"}
    b2r = {m: nc.dram_tensor(f"b2r_{m}", [1, D], f32, kind="ExternalInput") for m in "qk"}
    mask = nc.dram_tensor("mask", [P, D], bf16, kind="ExternalInput")
    dom = {m: nc.dram_tensor(f"dom_{m}", [NS, D], f32, kind="ExternalOutput") for m in "qk"}

    # ---- internal DRAM (collective bounce buffers) ----
    s_part = [nc.dram_tensor(f"s_part{j}", [D, 512], bf16) for j in range(2)]
    s_red = [nc.dram_tensor(f"s_red{j}", [P, 512], bf16) for j in range(2)]
    scb = nc.dram_tensor("scb", [P, D], bf16)
    sc_full = nc.dram_tensor("sc_full", [D, D], bf16, addr_space="Shared")
    m1s = {(m, h): nc.dram_tensor(f"m1s_{m}{h}", [D, HS // 2], bf16)
           for m in "qk" for h in range(2)}
    m1f = {(m, h): nc.dram_tensor(f"m1f_{m}{h}", [NCORES, D, HS // 2], bf16,
                                  addr_space="Shared")
           for m in "qk" for h in range(2)}

    RG = [list(range(NCORES))]
    NB = NS // P              # 32 n-blocks per core
    IT = D // P               # 8 feature tiles
    JW = 512                  # matmul moving free dim
    JH = D // JW              # 2 j-halves of S
    HB = H // P               # 32 hidden blocks
    KO = 4                    # phase-1 k-stream chunks (of NB//KO n-blocks each)
    NBC = NB // KO            # 8 n-blocks per stream chunk

    with tile.TileContext(nc) as tc:
        with (
            tc.tile_pool(name="small", bufs=1) as small,
            tc.tile_pool(name="dout", bufs=4) as doutp,
            tc.tile_pool(name="wpool", bufs=1) as wpool,
        ):
            mask_sb = small.tile([P, D], bf16)
            w2_tiles = {}
            w2_tiles["q"] = wpool.tile([P, HB, D], bf16, tag="w2big", name="w2_q")

            # ================= phase 1: S_partial = Qc^T Kc =================
            smx_cm = tc.tile_pool(name="smx", bufs=1)
            smx = smx_cm.__enter__()
            e2h, zh = [], []
            with (
                tc.tile_pool(name="ph1", bufs=1) as ph1,
                tc.tile_pool(name="kstream", bufs=2) as kstream,
                tc.tile_pool(name="ph1psum", bufs=1, space="PSUM") as ph1psum,
            ):
                q_ch = {}
                for jh in range(JH):
                    ps = [
                        ph1psum.tile([P, JW], f32, tag=f"sps{i}", name=f"sps{i}_{jh}")
                        for i in range(IT)
                    ]
                    for ko in range(KO):
                        if ko not in q_ch:
                            qc = ph1.tile([P, NBC, D], bf16, tag=f"qc{ko}",
                                          name=f"qc{ko}")
                            nc.sync.dma_start(
                                out=qc[:],
                                in_=q.ap()[ko * NBC * P:(ko + 1) * NBC * P, :]
                                    .rearrange("(nb p) d -> p nb d", p=P),
                            )
                            q_ch[ko] = qc
                        k_sb = kstream.tile([P, NBC, JW], bf16, tag="kc")
                        nc.sync.dma_start(
                            out=k_sb[:],
                            in_=k.ap()[ko * NBC * P:(ko + 1) * NBC * P,
                                       jh * JW:(jh + 1) * JW]
                                .rearrange("(nb p) d -> p nb d", p=P),
                        )
                        # trickle-load mask + next MLP's w2 behind the
                        # phase-1 operand stream
                        idx = jh * KO + ko
                        if idx == 0:
                            nc.sync.dma_start(out=mask_sb[:], in_=mask.ap())
                        nc.sync.dma_start(
                            out=w2_tiles["q"][:, idx * (HB // 8):(idx + 1) * (HB // 8), :],
                            in_=w2["q"].ap()
                                .rearrange("(hb p) d -> p hb d", p=P)[
                                    :, idx * (HB // 8):(idx + 1) * (HB // 8), :],
                        )
                        for nb in range(NBC):
                            for i in range(IT):
                                nc.tensor.matmul(
                                    ps[i][:],
                                    q_ch[ko][:, nb, i * P:(i + 1) * P],
                                    k_sb[:, nb, :],
                                    start=(ko == 0 and nb == 0),
                                    stop=(ko == KO - 1 and nb == NBC - 1),
                                )
                    for i in range(IT):
                        so = doutp.tile([P, JW], bf16, tag="sout")
                        nc.vector.tensor_copy(out=so[:], in_=ps[i][:])
                        nc.sync.dma_start(
                            out=s_part[jh].ap()[i * P:(i + 1) * P, :],
                            in_=so[:],
                        )
                    # ReduceScatter this column-half; the jh=0 one overlaps
                    # the jh=1 matmuls.
                    nc.gpsimd.collective_compute(
                        "ReduceScatter", add, replica_groups=RG,
                        ins=[s_part[jh].ap().opt()], outs=[s_red[jh].ap().opt()],
                    )
                    # softmax front half: runs as soon as this RS lands,
                    # overlapping the other half's matmuls / RS.
                    sred = smx.tile([P, JW], bf16, tag=f"sred{jh}", name=f"sred{jh}")
                    nc.sync.dma_start(out=sred[:], in_=s_red[jh].ap())
                    tm = smx.tile([P, JW], f32, tag=f"tm{jh}", name=f"tm{jh}")
                    nc.vector.tensor_tensor(
                        out=tm[:], in0=sred[:],
                        in1=mask_sb[:, jh * JW:(jh + 1) * JW], op=mult)
                    lg = smx.tile([P, JW], f32, tag=f"lg{jh}", name=f"lg{jh}")
                    nc.scalar.activation(out=lg[:], in_=tm[:], func=Exp)
                    e2 = smx.tile([P, JW], f32, tag=f"e2{jh}", name=f"e2{jh}")
                    zz = smx.tile([P, 1], f32, tag=f"z{jh}", name=f"z{jh}")
                    nc.scalar.activation(out=e2[:], in_=lg[:], func=Exp,
                                         accum_out=zz[:])
                    e2h.append(e2)
                    zh.append(zz)

            # ================= softmax merge tail =================
            zsum = smx.tile([P, 1], f32)
            nc.vector.tensor_tensor(out=zsum[:], in0=zh[0][:], in1=zh[1][:], op=add)
            rz = smx.tile([P, 1], f32)
            nc.vector.reciprocal(rz[:], zsum[:])
            scb_sb = smx.tile([P, D], bf16)
            for j in range(2):
                nc.vector.tensor_scalar(out=scb_sb[:, j * JW:(j + 1) * JW],
                                        in0=e2h[j][:], scalar1=rz[:],
                                        scalar2=None, op0=mult)
            nc.sync.dma_start(out=scb.ap(), in_=scb_sb[:])
            smx_cm.__exit__(None, None, None)

            nc.gpsimd.collective_compute(
                "AllGather", mybir.AluOpType.bypass, replica_groups=RG,
                ins=[scb.ap().opt()], outs=[sc_full.ap().opt()],
            )

            # ================= M1 = scores^T @ W1 (h-shard) =================
            with (
                tc.tile_pool(name="m1pool", bufs=1) as m1pool,
                tc.tile_pool(name="m1psum", bufs=3, space="PSUM") as m1psum,
            ):
                sc_sb = m1pool.tile([P, IT, D], bf16)
                nc.sync.dma_start(
                    out=sc_sb[:],
                    in_=sc_full.ap().rearrange("(it p) j -> p it j", p=P),
                )
                for m in "qk":
                    w1_sb = m1pool.tile([P, IT, HS], bf16, tag=f"w1_{m}")
                    nc.sync.dma_start(
                        out=w1_sb[:],
                        in_=w1s[m].ap().rearrange("(it p) h -> p it h", p=P),
                    )
                    for jm in range(IT):
                        mp = m1psum.tile([P, HS], f32, tag="m1ps",
                                         name=f"mp_{m}{jm}")
                        for it in range(IT):
                            nc.tensor.matmul(
                                mp[:],
                                sc_sb[:, it, jm * P:(jm + 1) * P],
                                w1_sb[:, it, :],
                                start=(it == 0),
                                stop=(it == IT - 1),
                            )
                        mo = doutp.tile([P, HS], bf16, tag="m1out",
                                        name=f"mo_{m}{jm}")
                        nc.vector.tensor_copy(out=mo[:], in_=mp[:])
                        for half in range(2):
                            nc.sync.dma_start(
                                out=m1s[m, half].ap()[jm * P:(jm + 1) * P, :],
                                in_=mo[:, half * (HS // 2):(half + 1) * (HS // 2)],
                            )
                    for half in range(2):
                        nc.gpsimd.collective_compute(
                            "AllGather", mybir.AluOpType.bypass, replica_groups=RG,
                            ins=[m1s[m, half].ap().opt()],
                            outs=[m1f[m, half].ap().opt()],
                        )

            # ================= MLPs =================
            with (
                tc.tile_pool(name="mlp", bufs=1) as mlp,
                tc.tile_pool(name="vstream", bufs=2) as vstream,
                tc.tile_pool(name="mlppsum", bufs=5, space="PSUM") as bpsum,
                tc.tile_pool(name="cpsum", bufs=3, space="PSUM") as cpsum,
            ):
                for m in "qk":
                    HH = HS // 2
                    m1_half = []
                    for half in range(2):
                        mh = mlp.tile([P, IT, H // 2], bf16, tag=f"m1big{half}",
                                      name=f"m1big{half}_{m}")
                        for c2 in range(NCORES):
                            nc.sync.dma_start(
                                out=mh[:, :, c2 * HH:(c2 + 1) * HH],
                                in_=m1f[m, half].ap()[c2]
                                    .rearrange("(jb p) h -> p jb h", p=P),
                            )
                        m1_half.append(mh)
                    hb_order = [hb for hb in range(HB) if (hb % 4) < 2] + \
                               [hb for hb in range(HB) if (hb % 4) >= 2]
                    if m in w2_tiles:
                        w2_sb = w2_tiles[m]
                    else:
                        w2_sb = wpool.tile([P, HB, D], bf16, tag="w2big",
                                           name=f"w2_{m}")
                        nc.sync.dma_start(
                            out=w2_sb[:],
                            in_=w2[m].ap().rearrange("(hb p) d -> p hb d", p=P),
                        )
                    b1_sb = small.tile([P, H // P], f32, tag="b1t")
                    nc.sync.dma_start(out=b1_sb[:], in_=b1t[m].ap())
                    b2_sb = small.tile([P, D], f32, tag="b2r")
                    b2_bcast = b2r[m].ap()
                    nc.sync.dma_start(
                        out=b2_sb[:],
                        in_=bass.AP(tensor=b2_bcast.tensor, offset=b2_bcast.offset,
                                    ap=[[0, P], *b2_bcast.ap[1:]]),
                    )

                    for ncnk in range(NS // JW):      # 8 chunks of 512 samples
                        vt_sb = vstream.tile([P, IT, JW], bf16, tag="vt")
                        nc.sync.dma_start(
                            out=vt_sb[:],
                            in_=vt.ap()[:, ncnk * JW:(ncnk + 1) * JW]
                                .rearrange("(jb p) n -> p jb n", p=P),
                        )
                        hid_sb = mlp.tile([P, HB, JW], bf16, tag="hid")
                        # hiddenT[h, n] = relu(sum_j M1[j,h] vT[j,n] + b1[h])
                        for hb in hb_order:
                            c2, pos = hb // 4, hb % 4
                            half, hh = pos // 2, pos % 2
                            off = c2 * HH + hh * P
                            pb = bpsum.tile([P, JW], f32, tag="psB")
                            for jb in range(IT):
                                nc.tensor.matmul(
                                    pb[:],
                                    m1_half[half][:, jb, off:off + P],
                                    vt_sb[:, jb, :],
                                    start=(jb == 0),
                                    stop=(jb == IT - 1),
                                )
                            nc.vector.tensor_scalar(
                                out=hid_sb[:, hb, :], in0=pb[:],
                                scalar1=b1_sb[:, hb:hb + 1], scalar2=0.0,
                                op0=add, op1=mx,
                            )
                        # dom[n, i2] = sum_h hidden[n,h] W2[h,i2] + b2[i2]
                        for ns in range(JW // P):     # 4 sample sub-tiles
                            for ih in range(JH):      # 2 output column halves
                                pc = cpsum.tile([P, JW], f32, tag="psC")
                                for hb in range(HB):
                                    nc.tensor.matmul(
                                        pc[:],
                                        hid_sb[:, hb, ns * P:(ns + 1) * P],
                                        w2_sb[:, hb, ih * JW:(ih + 1) * JW],
                                        start=(hb == 0), stop=(hb == HB - 1),
                                    )
                                do = doutp.tile([P, JW], f32, tag="dmout")
                                nc.vector.tensor_tensor(
                                    out=do[:], in0=pc[:],
                                    in1=b2_sb[:, ih * JW:(ih + 1) * JW],
                                    op=add,
                                )
                                nc.sync.dma_start(
                                    out=dom[m].ap()[
                                        ncnk * JW + ns * P:ncnk * JW + (ns + 1) * P,
                                        ih * JW:(ih + 1) * JW],
                                    in_=do[:],
                                )

    nc.compile()
    return nc


def _get_nc():
    if "nc" not in _CACHE:
        _CACHE["nc"] = _build()
    return _CACHE["nc"]


def _make_in_maps(inputs):
    query = np.asarray(inputs["query"])
    key = np.asarray(inputs["key"])
    value = np.asarray(inputs["value"])

    q_bf = query.astype(BF)
    k_bf = key.astype(BF)
    vt_bf = np.ascontiguousarray(value.T).astype(BF)          # [D, N]
    w1 = {"q": inputs["wq1"], "k": inputs["wk1"]}
    w2 = {"q": inputs["wq2"], "k": inputs["wk2"]}
    b1 = {"q": inputs["bq1"], "k": inputs["bk1"]}
    b2 = {"q": inputs["bq2"], "k": inputs["bk2"]}
    w1_bf = {m: np.asarray(w1[m]).astype(BF) for m in "qk"}
    w2_bf = {m: np.ascontiguousarray(np.asarray(w2[m]).astype(BF)) for m in "qk"}
    b1_t = {m: np.ascontiguousarray(
        np.asarray(b1[m]).astype(np.float32).reshape(H // P, P).T) for m in "qk"}
    b2_r = {m: np.asarray(b2[m]).astype(np.float32).reshape(1, D) for m in "qk"}

    in_maps = []
    diag = 1.0 - 1.0 / np.sqrt(D).astype(np.float32)
    for c in range(NCORES):
        msk = np.ones((P, D), np.float32)
        msk[np.arange(P), c * P + np.arange(P)] = diag
        msk = msk.astype(BF)
        im = {
            "q": np.ascontiguousarray(q_bf[c * NS:(c + 1) * NS]),
            "k": np.ascontiguousarray(k_bf[c * NS:(c + 1) * NS]),
            "vt": np.ascontiguousarray(vt_bf[:, c * NS:(c + 1) * NS]),
            "mask": msk,
        }
        for m in "qk":
            im[f"w1s_{m}"] = np.ascontiguousarray(
                w1_bf[m][:, c * HS:(c + 1) * HS])
            im[f"w2_{m}"] = w2_bf[m]
            im[f"b1t_{m}"] = b1_t[m]
            im[f"b2r_{m}"] = b2_r[m]
        in_maps.append(im)
    return in_maps


def _gather(results):
    dom_q = np.concatenate([results[c]["dom_q"] for c in range(NCORES)], axis=0)
    dom_k = np.concatenate([results[c]["dom_k"] for c in range(NCORES)], axis=0)
    return dom_q, dom_k


def _run(inputs, **kw):
    from concourse import bass_utils
    nc = _get_nc()
    in_maps = _make_in_maps(inputs)
    return bass_utils.run_bass_kernel_spmd(
        nc, in_maps, core_ids=list(range(NCORES)), **kw
    )


def kernel(**inputs):
    res = _run(inputs)
    return _gather(res.results)


# revision 17
# speedup vs baseline: 1.0349x; 1.0013x over previous
"""Trainium2 Bass kernel for nn_DomainAdaptation (sparse feature-attention + dual MLP).

Math (reference):
    S = Q^T K                        [D, D], contraction over N
    L = exp(S - S*I/sqrt(D))
    scores = softmax(L, axis=-1)
    attn = (scores @ V^T)^T          [N, D]
    dom_q = relu(attn @ Wq1 + bq1) @ Wq2 + bq2
    dom_k = relu(attn @ Wk1 + bk1) @ Wk2 + bk2

Key restructuring: attn = V @ scores^T, so
    attn @ W1 = V @ (scores^T @ W1) = V @ M1
and attn is never materialized. Per core (N sharded 8 ways):
    phase 1: S_partial = Qc^T Kc  (bf16 matmuls, f32 accum)
    ReduceScatter(S) -> each core owns a 128-row block of S
    softmax block (f32, incl. double-exp of the reference), AllGather(scores)
    M1 = scores^T @ W1, h-sharded per core, AllGather(M1)  [x2 for q/k]
    MLP: hiddenT = relu(M1^T @ Vc^T + b1);  dom = hidden^T-contracted @ W2 + b2
outputs in natural [N, D] f32 orientation.
"""

import numpy as np
import ml_dtypes

N, D, H = 32768, 1024, 4096
NCORES = 8
NS = N // NCORES          # 4096 sample rows per core
HS = H // NCORES          # 512 hidden cols per core (M1 shard)
P = 128
BF = ml_dtypes.bfloat16

_CACHE: dict = {}


def _build():
    import concourse.bass as bass
    import concourse.tile as tile
    from concourse import bacc, mybir

    f32 = mybir.dt.float32
    bf16 = mybir.dt.bfloat16
    Exp = mybir.ActivationFunctionType.Exp
    add = mybir.AluOpType.add
    mx = mybir.AluOpType.max
    mult = mybir.AluOpType.mult

    nc = bacc.Bacc("TRN2", target_bir_lowering=False, debug=False, num_devices=NCORES)

    # ---- I/O ----
    q = nc.dram_tensor("q", [NS, D], bf16, kind="ExternalInput")
    k = nc.dram_tensor("k", [NS, D], bf16, kind="ExternalInput")
    vt = nc.dram_tensor("vt", [D, NS], bf16, kind="ExternalInput")
    w1s = {m: nc.dram_tensor(f"w1s_{m}", [D, HS], bf16, kind="ExternalInput") for m in "qk"}
    w2 = {m: nc.dram_tensor(f"w2_{m}", [H, D], bf16, kind="ExternalInput") for m in "qk"}
    b1t = {m: nc.dram_tensor(f"b1t_{m}", [P, H // P], f32, kind="ExternalInput") for m in "qk"}
    b2r = {m: nc.dram_tensor(f"b2r_{m}", [1, D], f32, kind="ExternalInput") for m in "qk"}
    mask = nc.dram_tensor("mask", [P, D], bf16, kind="ExternalInput")
    dom = {m: nc.dram_tensor(f"dom_{m}", [NS, D], f32, kind="ExternalOutput") for m in "qk"}

    # ---- internal DRAM (collective bounce buffers) ----
    s_part = [nc.dram_tensor(f"s_part{j}", [D, 512], bf16) for j in range(2)]
    s_red = [nc.dram_tensor(f"s_red{j}", [P, 512], bf16) for j in range(2)]
    scb = nc.dram_tensor("scb", [P, D], bf16)
    sc_full = nc.dram_tensor("sc_full", [D, D], bf16, addr_space="Shared")
    m1s = {(m, h): nc.dram_tensor(f"m1s_{m}{h}", [D, HS // 2], bf16)
           for m in "qk" for h in range(2)}
    m1f = {(m, h): nc.dram_tensor(f"m1f_{m}{h}", [NCORES, D, HS // 2], bf16,
                                  addr_space="Shared")
           for m in "qk" for h in range(2)}

    RG = [list(range(NCORES))]
    NB = NS // P              # 32 n-blocks per core
    IT = D // P               # 8 feature tiles
    JW = 512                  # matmul moving free dim
    JH = D // JW              # 2 j-halves of S
    HB = H // P               # 32 hidden blocks
    KO = 4                    # phase-1 k-stream chunks (of NB//KO n-blocks each)
    NBC = NB // KO            # 8 n-blocks per stream chunk

    with tile.TileContext(nc) as tc:
        with (
            tc.tile_pool(name="small", bufs=1) as small,
            tc.tile_pool(name="dout", bufs=4) as doutp,
            tc.tile_pool(name="wpool", bufs=1) as wpool,
        ):
            mask_sb = small.tile([P, D], bf16)
            w2_tiles = {}
            w2_tiles["q"] = wpool.tile([P, HB, D], bf16, tag="w2big", name="w2_q")

            # ================= phase 1: S_partial = Qc^T Kc =================
            smx_cm = tc.tile_pool(name="smx", bufs=1)
            smx = smx_cm.__enter__()
            e2h, zh = [], []
            with (
                tc.tile_pool(name="ph1", bufs=1) as ph1,
                tc.tile_pool(name="kstream", bufs=2) as kstream,
                tc.tile_pool(name="ph1psum", bufs=1, space="PSUM") as ph1psum,
            ):
                q_ch = {}
                for jh in range(JH):
                    ps = [
                        ph1psum.tile([P, JW], f32, tag=f"sps{i}", name=f"sps{i}_{jh}")
                        for i in range(IT)
                    ]
                    for ko in range(KO):
                        if ko not in q_ch:
                            qc = ph1.tile([P, NBC, D], bf16, tag=f"qc{ko}",
                                          name=f"qc{ko}")
                            nc.sync.dma_start(
                                out=qc[:],
                                in_=q.ap()[ko * NBC * P:(ko + 1) * NBC * P, :]
                                    .rearrange("(nb p) d -> p nb d", p=P),
                            )
                            q_ch[ko] = qc
                        k_sb = kstream.tile([P, NBC, JW], bf16, tag="kc")
                        nc.sync.dma_start(
                            out=k_sb[:],
                            in_=k.ap()[ko * NBC * P:(ko + 1) * NBC * P,
                                       jh * JW:(jh + 1) * JW]
                                .rearrange("(nb p) d -> p nb d", p=P),
                        )
                        # trickle-load mask + next MLP's w2 behind the
                        # phase-1 operand stream
                        idx = jh * KO + ko
                        if idx == 0:
                            nc.sync.dma_start(out=mask_sb[:], in_=mask.ap())
                        nc.sync.dma_start(
                            out=w2_tiles["q"][:, idx * (HB // 8):(idx + 1) * (HB // 8), :],
                            in_=w2["q"].ap()
                                .rearrange("(hb p) d -> p hb d", p=P)[
                                    :, idx * (HB // 8):(idx + 1) * (HB // 8), :],
                        )
                        for nb in range(NBC):
                            for i in range(IT):
                                nc.tensor.matmul(
                                    ps[i][:],
                                    q_ch[ko][:, nb, i * P:(i + 1) * P],
                                    k_sb[:, nb, :],
                                    start=(ko == 0 and nb == 0),
                                    stop=(ko == KO - 1 and nb == NBC - 1),
                                )
                    for i in range(IT):
                        so = doutp.tile([P, JW], bf16, tag="sout")
                        nc.vector.tensor_copy(out=so[:], in_=ps[i][:])
                        nc.sync.dma_start(
                            out=s_part[jh].ap()[i * P:(i + 1) * P, :],
                            in_=so[:],
                        )
                    # ReduceScatter this column-half; the jh=0 one overlaps
                    # the jh=1 matmuls.
                    nc.gpsimd.collective_compute(
                        "ReduceScatter", add, replica_groups=RG,
                        ins=[s_part[jh].ap().opt()], outs=[s_red[jh].ap().opt()],
                    )
                    # softmax front half: runs as soon as this RS lands,
                    # overlapping the other half's matmuls / RS.
                    sred = smx.tile([P, JW], bf16, tag=f"sred{jh}", name=f"sred{jh}")
                    nc.sync.dma_start(out=sred[:], in_=s_red[jh].ap())
                    tm = smx.tile([P, JW], f32, tag=f"tm{jh}", name=f"tm{jh}")
                    nc.vector.tensor_tensor(
                        out=tm[:], in0=sred[:],
                        in1=mask_sb[:, jh * JW:(jh + 1) * JW], op=mult)
                    lg = smx.tile([P, JW], f32, tag=f"lg{jh}", name=f"lg{jh}")
                    nc.scalar.activation(out=lg[:], in_=tm[:], func=Exp)
                    e2 = smx.tile([P, JW], f32, tag=f"e2{jh}", name=f"e2{jh}")
                    zz = smx.tile([P, 1], f32, tag=f"z{jh}", name=f"z{jh}")
                    nc.scalar.activation(out=e2[:], in_=lg[:], func=Exp,
                                         accum_out=zz[:])
                    e2h.append(e2)
                    zh.append(zz)

            # ================= softmax merge tail =================
            zsum = smx.tile([P, 1], f32)
            nc.vector.tensor_tensor(out=zsum[:], in0=zh[0][:], in1=zh[1][:], op=add)
            rz = smx.tile([P, 1], f32)
            nc.vector.reciprocal(rz[:], zsum[:])
            scb_sb = smx.tile([P, D], bf16)
            for j in range(2):
                nc.vector.tensor_scalar(out=scb_sb[:, j * JW:(j + 1) * JW],
                                        in0=e2h[j][:], scalar1=rz[:],
                                        scalar2=None, op0=mult)
            nc.sync.dma_start(out=scb.ap(), in_=scb_sb[:])
            smx_cm.__exit__(None, None, None)

            nc.gpsimd.collective_compute(
                "AllGather", mybir.AluOpType.bypass, replica_groups=RG,
                ins=[scb.ap().opt()], outs=[sc_full.ap().opt()],
            )

            # ================= M1 = scores^T @ W1 (h-shard) =================
            with (
                tc.tile_pool(name="m1pool", bufs=1) as m1pool,
                tc.tile_pool(name="m1psum", bufs=3, space="PSUM") as m1psum,
            ):
                sc_t = []
                for it in range(IT):
                    sct = m1pool.tile([P, D], bf16, tag=f"sc{it}", name=f"sc{it}")
                    nc.sync.dma_start(
                        out=sct[:],
                        in_=sc_full.ap()[it * P:(it + 1) * P, :],
                    )
                    sc_t.append(sct)
                for m in "qk":
                    w1_sb = m1pool.tile([P, IT, HS], bf16, tag=f"w1_{m}")
                    nc.sync.dma_start(
                        out=w1_sb[:],
                        in_=w1s[m].ap().rearrange("(it p) h -> p it h", p=P),
                    )
                    for jm in range(IT):
                        mp = m1psum.tile([P, HS], f32, tag="m1ps",
                                         name=f"mp_{m}{jm}")
                        for it in range(IT):
                            nc.tensor.matmul(
                                mp[:],
                                sc_t[it][:, jm * P:(jm + 1) * P],
                                w1_sb[:, it, :],
                                start=(it == 0),
                                stop=(it == IT - 1),
                            )
                        mo = doutp.tile([P, HS], bf16, tag="m1out",
                                        name=f"mo_{m}{jm}")
                        nc.vector.tensor_copy(out=mo[:], in_=mp[:])
                        for half in range(2):
                            nc.sync.dma_start(
                                out=m1s[m, half].ap()[jm * P:(jm + 1) * P, :],
                                in_=mo[:, half * (HS // 2):(half + 1) * (HS // 2)],
                            )
                    for half in range(2):
                        nc.gpsimd.collective_compute(
                            "AllGather", mybir.AluOpType.bypass, replica_groups=RG,
                            ins=[m1s[m, half].ap().opt()],
                            outs=[m1f[m, half].ap().opt()],
                        )

            # ================= MLPs =================
            with (
                tc.tile_pool(name="mlp", bufs=1) as mlp,
                tc.tile_pool(name="vstream", bufs=2) as vstream,
                tc.tile_pool(name="mlppsum", bufs=5, space="PSUM") as bpsum,
                tc.tile_pool(name="cpsum", bufs=3, space="PSUM") as cpsum,
            ):
                for m in "qk":
                    HH = HS // 2
                    m1_half = []
                    for half in range(2):
                        row = []
                        for c2 in range(NCORES):
                            mt = mlp.tile([P, IT, HH], bf16,
                                          tag=f"m1big{half}_{c2}",
                                          name=f"m1t{half}_{c2}_{m}")
                            nc.sync.dma_start(
                                out=mt[:],
                                in_=m1f[m, half].ap()[c2]
                                    .rearrange("(jb p) h -> p jb h", p=P),
                            )
                            row.append(mt)
                        m1_half.append(row)
                    hb_order = [hb for hb in range(HB) if (hb % 4) < 2] + \
                               [hb for hb in range(HB) if (hb % 4) >= 2]
                    if m in w2_tiles:
                        w2_sb = w2_tiles[m]
                    else:
                        w2_sb = wpool.tile([P, HB, D], bf16, tag="w2big",
                                           name=f"w2_{m}")
                        nc.sync.dma_start(
                            out=w2_sb[:],
                            in_=w2[m].ap().rearrange("(hb p) d -> p hb d", p=P),
                        )
                    b1_sb = small.tile([P, H // P], f32, tag="b1t")
                    nc.sync.dma_start(out=b1_sb[:], in_=b1t[m].ap())
                    b2_sb = small.tile([P, D], f32, tag="b2r")
                    b2_bcast = b2r[m].ap()
                    nc.sync.dma_start(
                        out=b2_sb[:],
                        in_=bass.AP(tensor=b2_bcast.tensor, offset=b2_bcast.offset,
                                    ap=[[0, P], *b2_bcast.ap[1:]]),
                    )

                    for ncnk in range(NS // JW):      # 8 chunks of 512 samples
                        vt_sb = vstream.tile([P, IT, JW], bf16, tag="vt")
                        nc.sync.dma_start(
                            out=vt_sb[:],
                            in_=vt.ap()[:, ncnk * JW:(ncnk + 1) * JW]
                                .rearrange("(jb p) n -> p jb n", p=P),
                        )
                        hid_sb = mlp.tile([P, HB, JW], bf16, tag="hid")
                        # hiddenT[h, n] = relu(sum_j M1[j,h] vT[j,n] + b1[h])
                        for hb in hb_order:
                            c2, pos = hb // 4, hb % 4
                            half, hh = pos // 2, pos % 2
                            pb = bpsum.tile([P, JW], f32, tag="psB")
                            for jb in range(IT):
                                nc.tensor.matmul(
                                    pb[:],
                                    m1_half[half][c2][:, jb, hh * P:(hh + 1) * P],
                                    vt_sb[:, jb, :],
                                    start=(jb == 0),
                                    stop=(jb == IT - 1),
                                )
                            nc.vector.tensor_scalar(
                                out=hid_sb[:, hb, :], in0=pb[:],
                                scalar1=b1_sb[:, hb:hb + 1], scalar2=0.0,
                                op0=add, op1=mx,
                            )
                        # dom[n, i2] = sum_h hidden[n,h] W2[h,i2] + b2[i2]
                        for ns in range(JW // P):     # 4 sample sub-tiles
                            for ih in range(JH):      # 2 output column halves
                                pc = cpsum.tile([P, JW], f32, tag="psC")
                                for hb in range(HB):
                                    nc.tensor.matmul(
                                        pc[:],
                                        hid_sb[:, hb, ns * P:(ns + 1) * P],
                                        w2_sb[:, hb, ih * JW:(ih + 1) * JW],
                                        start=(hb == 0), stop=(hb == HB - 1),
                                    )
                                do = doutp.tile([P, JW], f32, tag="dmout")
                                nc.vector.tensor_tensor(
                                    out=do[:], in0=pc[:],
                                    in1=b2_sb[:, ih * JW:(ih + 1) * JW],
                                    op=add,
                                )
                                nc.sync.dma_start(
                                    out=dom[m].ap()[
                                        ncnk * JW + ns * P:ncnk * JW + (ns + 1) * P,
                                        ih * JW:(ih + 1) * JW],
                                    in_=do[:],
                                )

    nc.compile()
    return nc


def _get_nc():
    if "nc" not in _CACHE:
        _CACHE["nc"] = _build()
    return _CACHE["nc"]


def _make_in_maps(inputs):
    query = np.asarray(inputs["query"])
    key = np.asarray(inputs["key"])
    value = np.asarray(inputs["value"])

    q_bf = query.astype(BF)
    k_bf = key.astype(BF)
    vt_bf = np.ascontiguousarray(value.T).astype(BF)          # [D, N]
    w1 = {"q": inputs["wq1"], "k": inputs["wk1"]}
    w2 = {"q": inputs["wq2"], "k": inputs["wk2"]}
    b1 = {"q": inputs["bq1"], "k": inputs["bk1"]}
    b2 = {"q": inputs["bq2"], "k": inputs["bk2"]}
    w1_bf = {m: np.asarray(w1[m]).astype(BF) for m in "qk"}
    w2_bf = {m: np.ascontiguousarray(np.asarray(w2[m]).astype(BF)) for m in "qk"}
    b1_t = {m: np.ascontiguousarray(
        np.asarray(b1[m]).astype(np.float32).reshape(H // P, P).T) for m in "qk"}
    b2_r = {m: np.asarray(b2[m]).astype(np.float32).reshape(1, D) for m in "qk"}

    in_maps = []
    diag = 1.0 - 1.0 / np.sqrt(D).astype(np.float32)
    for c in range(NCORES):
        msk = np.ones((P, D), np.float32)
        msk[np.arange(P), c * P + np.arange(P)] = diag
        msk = msk.astype(BF)
        im = {
            "q": np.ascontiguousarray(q_bf[c * NS:(c + 1) * NS]),
            "k": np.ascontiguousarray(k_bf[c * NS:(c + 1) * NS]),
            "vt": np.ascontiguousarray(vt_bf[:, c * NS:(c + 1) * NS]),
            "mask": msk,
        }
        for m in "qk":
            im[f"w1s_{m}"] = np.ascontiguousarray(
                w1_bf[m][:, c * HS:(c + 1) * HS])
            im[f"w2_{m}"] = w2_bf[m]
            im[f"b1t_{m}"] = b1_t[m]
            im[f"b2r_{m}"] = b2_r[m]
        in_maps.append(im)
    return in_maps


def _gather(results):
    dom_q = np.concatenate([results[c]["dom_q"] for c in range(NCORES)], axis=0)
    dom_k = np.concatenate([results[c]["dom_k"] for c in range(NCORES)], axis=0)
    return dom_q, dom_k


def _run(inputs, **kw):
    from concourse import bass_utils
    nc = _get_nc()
    in_maps = _make_in_maps(inputs)
    return bass_utils.run_bass_kernel_spmd(
        nc, in_maps, core_ids=list(range(NCORES)), **kw
    )


def kernel(**inputs):
    res = _run(inputs)
    return _gather(res.results)
